# revision 1
# baseline (speedup 1.0000x reference)
"""Trainium2 Bass kernel for nn_Block_71665824301263 (GNN message passing block).

Computation (see reference): BatchNorm -> TransformerConv-style edge attention
(6 heads, edge features added to K and V, segment softmax over incoming edges)
-> skip + residual -> BatchNorm -> MLP (gelu) -> residual.

Distribution over 8 NeuronCores:
- nodes sharded 2500/core; incoming edges partitioned by dst and sorted by dst
- weights replicated
- k/v projections AllGather'ed (bf16) so every core can gather src rows
- BatchNorm statistics AllReduce'd (2x [384] sums per BN)

Per-core edge stream is padded per 128-node block to a common tile count
(K[b] = max over cores), so all 8 cores run one identical program (SPMD).
Per-tile work: e = edge_attr @ We into PSUM; ke = k[src]+e (DVE); ve = v[src]+e
(identity-matmul accumulate); logits = segmented reduce of q[dst]*ke;
w = exp(0.125*logits) (ACT); agg/denoms via one-hot S matmul into per-block
PSUM accumulators.
"""

import os
import numpy as np
import ml_dtypes

import concourse.bass as bass
import concourse.bacc as bacc
import concourse.tile as tile
import concourse.mybir as mybir
from concourse.bass_utils import run_bass_kernel_spmd
from concourse.masks import make_identity

C = 8            # cores
N = 20000        # nodes
NL = N // C      # nodes per core
D = 384
H = 6
DHEAD = 64
DH = 2 * D       # mlp hidden
P = 128
NBLK = (NL + P - 1) // P      # 20 node blocks per core (last has 68)
TF = 3                        # DVE fuse factor (tiles per super)
SCALE = 1.0 / np.sqrt(np.float32(DHEAD))
EPS = 1e-5

f32 = mybir.dt.float32
bf16 = mybir.dt.bfloat16
i32 = mybir.dt.int32
BF = ml_dtypes.bfloat16
AF = mybir.ActivationFunctionType
ALU = mybir.AluOpType


def _block_pb(b):
    return min(P, NL - b * P)


def _prep_host(x, edge_index, edge_attr, weights):
    """Shard + pad edges, build per-core input maps. Returns (in_maps, K, T)."""
    src = np.asarray(edge_index[0]).astype(np.int64)
    dst = np.asarray(edge_index[1]).astype(np.int64)
    x = np.asarray(x, dtype=np.float32)
    edge_attr = np.asarray(edge_attr, dtype=np.float32)

    cores = []
    cnt = np.zeros((C, NBLK), np.int64)
    for c in range(C):
        sel = (dst >= c * NL) & (dst < (c + 1) * NL)
        eids = np.nonzero(sel)[0]
        d_loc = (dst[eids] - c * NL).astype(np.int64)
        order = np.argsort(d_loc, kind="stable")
        eids = eids[order]
        d_loc = d_loc[order]
        s_glob = src[eids]
        blk = d_loc // P
        cnt[c] = np.bincount(blk, minlength=NBLK)
        cores.append((eids, d_loc, s_glob, blk))

    K = [max(1, int(-(-cnt[:, b].max() // P))) for b in range(NBLK)]
    T = sum(K)
    pad = (-T) % TF
    K[NBLK - 1] += pad
    T += pad
    tile_block = np.repeat(np.arange(NBLK), K)          # block id per tile
    blk_tile_start = np.concatenate([[0], np.cumsum(K)])[:NBLK]
    blk_edge_start = blk_tile_start * P

    # replicated weight tensors
    def chunks(w, nk):
        return np.stack([w[i * P:(i + 1) * P] for i in range(nk)]).astype(BF)

    (Wq, bq, Wk, bk, Wv, bv, We, Wskip, bskip,
     g1, b1, g2, b2, W1, bm1, W2, bm2) = weights

    def aug(w, b):
        a = np.zeros((4, P, w.shape[1]), np.float32)
        a[:3] = np.stack([w[i * P:(i + 1) * P] for i in range(3)])
        a[3, 0] = b
        return a.astype(BF)

    shared = {
        "Wq": aug(Wq, bq), "Wk": aug(Wk, bk), "Wv": aug(Wv, bv),
        "Wsk": aug(Wskip, bskip),
        "We": chunks(We, 3),
        "W1": aug(W1, bm1)[:3],                 # bias separately (bm1T)
        "bm1T": np.asarray(bm1, np.float32).reshape(H, P).T.copy(),
        "W2": chunks(W2, 6),
        "bm2": np.asarray(bm2, np.float32).reshape(1, D).astype(BF),
        "gb": np.stack([np.asarray(v, np.float32).reshape(3, P)
                        for v in (g1, b1, g2, b2)], axis=-1),  # [3, P, 4]
    }

    in_maps = []
    for c in range(C):
        eids, d_loc, s_glob, blk = cores[c]
        starts = np.searchsorted(blk, np.arange(NBLK))
        rank = np.arange(len(blk)) - starts[blk]
        pos = blk_edge_start[blk] + rank

        src_pad = np.zeros(T * P, np.int64)
        dst_pad = np.zeros(T * P, np.int64)
        valid = np.zeros(T * P, bool)
        src_pad[pos] = s_glob
        dst_pad[pos] = d_loc
        valid[pos] = True

        ea_pad = np.zeros((T * P, D), np.float32)
        ea_pad[pos] = edge_attr[eids]
        eaT = ea_pad.astype(BF).reshape(T, P, 3, P).transpose(0, 3, 2, 1)

        S = np.zeros((T * P, P), np.float32)
        tb = np.repeat(tile_block, P)
        S[np.nonzero(valid)[0], (dst_pad - tb * P)[valid]] = 1.0
        S = S.astype(BF).reshape(T, P, P)

        # combined [T, P, 5, P]: chunks 0-2 = eaT (partition=feature),
        # chunk 3 = S (partition=edge), chunk 4 = S^T (partition=node)
        ST = S.transpose(0, 2, 1)
        eaS = np.concatenate([eaT, S[:, :, None, :], ST[:, :, None, :]], axis=2)
        eaS = np.ascontiguousarray(eaS)

        kidx = np.where(valid, src_pad, 0)
        kidxT = np.ascontiguousarray(kidx.reshape(T, P).T).astype(np.int32)

        x_loc = np.ascontiguousarray(x[c * NL:(c + 1) * NL])
        xT_loc = np.ascontiguousarray(x_loc.T)

        m = {"x_loc": x_loc, "xT_loc": xT_loc, "eaS": eaS, "kidx": kidxT}
        m.update(shared)
        in_maps.append(m)
    return in_maps, K, T


def _build(K, T):
    PH = int(os.environ.get("KPH", "6"))
    NOCC = int(os.environ.get("KNOCC", "0"))
    nc = bacc.Bacc("TRN2", target_bir_lowering=False, debug=False,
                   enable_asserts=False, num_devices=C)
    tile_block = np.repeat(np.arange(NBLK), K)
    blk_tile_start = np.concatenate([[0], np.cumsum(K)])[:NBLK]

    # ------------- I/O -------------
    x_in = nc.dram_tensor("x_loc", [NL, D], f32, kind="ExternalInput")
    xT_in = nc.dram_tensor("xT_loc", [D, NL], f32, kind="ExternalInput")
    eaS_in = nc.dram_tensor("eaS", [T, P, 5, P], bf16, kind="ExternalInput")
    kidx_in = nc.dram_tensor("kidx", [P, T], i32, kind="ExternalInput")
    wq_in = nc.dram_tensor("Wq", [4, P, D], bf16, kind="ExternalInput")
    wk_in = nc.dram_tensor("Wk", [4, P, D], bf16, kind="ExternalInput")
    wv_in = nc.dram_tensor("Wv", [4, P, D], bf16, kind="ExternalInput")
    wsk_in = nc.dram_tensor("Wsk", [4, P, D], bf16, kind="ExternalInput")
    we_in = nc.dram_tensor("We", [3, P, D], bf16, kind="ExternalInput")
    w1_in = nc.dram_tensor("W1", [3, P, DH], bf16, kind="ExternalInput")
    bm1_in = nc.dram_tensor("bm1T", [P, H], f32, kind="ExternalInput")
    w2_in = nc.dram_tensor("W2", [6, P, D], bf16, kind="ExternalInput")
    bm2_in = nc.dram_tensor("bm2", [1, D], bf16, kind="ExternalInput")
    gb_in = nc.dram_tensor("gb", [3, P, 4], f32, kind="ExternalInput")
    out_dram = nc.dram_tensor("out", [NL, D], f32, kind="ExternalOutput")
    KDBG = int(os.environ.get("KDBG", "0"))
    if KDBG:
        dbg_dram = nc.dram_tensor("dbg", [NBLK * P, D + H], f32,
                                  kind="ExternalOutput")

    # ------------- internal DRAM -------------
    kv_part = nc.dram_tensor("kv_part", [NL, 2 * D], bf16)
    kv_full = nc.dram_tensor("kv_full", [C * NL, 2 * D], bf16,
                             addr_space="Shared")
    bn1_i = nc.dram_tensor("bn1_i", [P, 6], f32)
    bn1_o = nc.dram_tensor("bn1_o", [P, 6], f32, addr_space="Shared")
    bn2_i = nc.dram_tensor("bn2_i", [P, 6], f32)
    bn2_o = nc.dram_tensor("bn2_o", [P, 6], f32, addr_space="Shared")

    rg = [list(range(C))]

    with tile.TileContext(nc) as tc:
        with tc.tile_pool(name="const", bufs=1) as cp, \
             tc.tile_pool(name="persist", bufs=1) as pp, \
             tc.tile_pool(name="small", bufs=2) as sp:
            # ---- constants ----
            idn_f = cp.tile([P, P], f32, tag="idn_f")
            make_identity(nc, idn_f[:])
            idn_b = cp.tile([P, P], bf16, tag="idn_b")
            make_identity(nc, idn_b[:])
            ones_b = cp.tile([1, NL], bf16, tag="ones_b")
            nc.vector.memset(ones_b[:], 1.0)
            zc = cp.tile([P, 1], f32, tag="zc")
            nc.vector.memset(zc[:], 0.0)
            epst = cp.tile([P, 1], f32, tag="epst")
            nc.vector.memset(epst[:], EPS)
            wq_sb = cp.tile([P, 4, D], bf16, tag="wq")
            nc.sync.dma_start(wq_sb[:], wq_in.ap().rearrange("c p d -> p c d"))
            wk_sb = cp.tile([P, 4, D], bf16, tag="wk")
            nc.sync.dma_start(wk_sb[:], wk_in.ap().rearrange("c p d -> p c d"))
            wv_sb = cp.tile([P, 4, D], bf16, tag="wv")
            nc.sync.dma_start(wv_sb[:], wv_in.ap().rearrange("c p d -> p c d"))
            wsk_sb = cp.tile([P, 4, D], bf16, tag="wsk")
            nc.sync.dma_start(wsk_sb[:], wsk_in.ap().rearrange("c p d -> p c d"))
            we_sb = cp.tile([P, 3, D], bf16, tag="we")
            nc.sync.dma_start(we_sb[:], we_in.ap().rearrange("c p d -> p c d"))
            w1_sb = cp.tile([P, 3, DH], bf16, tag="w1")
            nc.sync.dma_start(w1_sb[:], w1_in.ap().rearrange("c p d -> p c d"))
            bm1_sb = cp.tile([P, H], f32, tag="bm1")
            nc.sync.dma_start(bm1_sb[:], bm1_in[:, :])
            w2_sb = cp.tile([P, 6, D], bf16, tag="w2")
            nc.sync.dma_start(w2_sb[:], w2_in.ap().rearrange("c p d -> p c d"))
            bm2_sb = cp.tile([1, D], bf16, tag="bm2")
            nc.sync.dma_start(bm2_sb[:], bm2_in[:, :])
            gb_sb = cp.tile([P, 3, 4], f32, tag="gb")
            nc.sync.dma_start(gb_sb[:], gb_in.ap().rearrange("c p j -> p c j"))

            # ---- BN1 stats (on xT, feature-major) ----
            hp_cm = tc.tile_pool(name="bnh", bufs=1)
            hp = hp_cm.__enter__()
            bp_cm = tc.tile_pool(name="bn1", bufs=1)
            bp = bp_cm.__enter__()
            xT = [bp.tile([P, NL], f32, tag=f"xT{c}", name=f"xT{c}") for c in range(3)]
            bn1_sb = cp.tile([P, 6], f32, tag="bn1sb")
            for c in range(3):
                nc.sync.dma_start(xT[c][:], xT_in[c * P:(c + 1) * P, :])
                nc.vector.tensor_reduce(
                    out=bn1_sb[:, 2 * c:2 * c + 1], in_=xT[c][:],
                    axis=mybir.AxisListType.X, op=ALU.add)
                sq = bp.tile([P, NL], f32, tag="sq_bn1", name="sq1", bufs=2)
                nc.vector.tensor_tensor(out=sq[:], in0=xT[c][:], in1=xT[c][:],
                                        op=ALU.mult)
                nc.vector.tensor_reduce(
                    out=bn1_sb[:, 2 * c + 1:2 * c + 2], in_=sq[:],
                    axis=mybir.AxisListType.X, op=ALU.add)
            nc.sync.dma_start(bn1_i[:, :], bn1_sb[:])
            if NOCC:
                nc.sync.dma_start(bn1_o[:, :], bn1_i[:, :])
            else:
                nc.gpsimd.collective_compute(
                    "AllReduce", ALU.add, replica_groups=rg,
                    ins=[bn1_i.ap().opt()], outs=[bn1_o.ap().opt()])
            st1 = cp.tile([P, 6], f32, tag="st1")
            nc.sync.dma_start(st1[:], bn1_o[:, :])

            def bn_affine(st, gcol, bcol, scn, bin_):
                """From AllReduced [P, 6] (sum, sumsq per chunk) compute
                scale/bias [P, 3] tiles."""
                sc_t = cp.tile([P, 3], f32, tag=scn)
                bi_t = cp.tile([P, 3], f32, tag=bin_)
                for c in range(3):
                    mean = sp.tile([P, 1], f32, tag="bn_mean")
                    nc.vector.tensor_scalar_mul(mean[:], st[:, 2 * c:2 * c + 1],
                                                1.0 / N)
                    var = sp.tile([P, 1], f32, tag="bn_var")
                    nc.vector.tensor_scalar_mul(var[:], st[:, 2 * c + 1:2 * c + 2],
                                                1.0 / N)
                    msq = sp.tile([P, 1], f32, tag="bn_msq")
                    nc.vector.tensor_tensor(out=msq[:], in0=mean[:], in1=mean[:],
                                            op=ALU.mult)
                    nc.vector.tensor_tensor(out=var[:], in0=var[:], in1=msq[:],
                                            op=ALU.subtract)
                    std = sp.tile([P, 1], f32, tag="bn_std")
                    nc.scalar.activation(std[:], var[:], AF.Sqrt, bias=epst[:, 0:1])
                    rstd = sp.tile([P, 1], f32, tag="bn_rstd")
                    nc.vector.reciprocal(rstd[:], std[:])
                    nc.vector.tensor_tensor(out=sc_t[:, c:c + 1], in0=rstd[:],
                                            in1=gb_sb[:, c, gcol:gcol + 1],
                                            op=ALU.mult)
                    ms = sp.tile([P, 1], f32, tag="bn_ms")
                    nc.vector.tensor_tensor(out=ms[:], in0=mean[:],
                                            in1=sc_t[:, c:c + 1], op=ALU.mult)
                    nc.vector.tensor_tensor(out=bi_t[:, c:c + 1],
                                            in0=gb_sb[:, c, bcol:bcol + 1],
                                            in1=ms[:], op=ALU.subtract)
                return sc_t, bi_t

            sc1, bi1 = bn_affine(st1, 0, 1, "sc1", "bi1")

            # hT = normalized x, bf16, feature-major
            hT = [hp.tile([P, NL], bf16, tag=f"hT{c}", name=f"hT{c}") for c in range(3)]
            for c in range(3):
                nc.scalar.activation(hT[c][:], xT[c][:], AF.Identity,
                                     scale=sc1[:, c:c + 1], bias=bi1[:, c:c + 1])
            bp_cm.__exit__(None, None, None)

            # ---- projections q,k,v,skip + base = x + skip ----
            q_sb = [pp.tile([P, D], bf16, tag=f"q{b}", name=f"q{b}")
                    for b in range(NBLK)]
            if PH < 2:
                nc.sync.dma_start(out_dram[0:P, 0:6], bn1_sb[:])
            base = [pp.tile([P, D], f32, tag=f"base{b}", name=f"base{b}") for b in range(NBLK)]
            if PH >= 2:
              with tc.tile_pool(name="proj", bufs=3) as jp, \
                   tc.tile_pool(name="projps", bufs=3, space="PSUM") as jpp:
                for b in range(NBLK):
                    pb = _block_pb(b)
                    ns = slice(b * P, b * P + pb)
                    for wsb, name in ((wq_sb, "q"), (wk_sb, "k"),
                                      (wv_sb, "v"), (wsk_sb, "s")):
                        ps = jpp.tile([P, D], f32, tag="proj_ps", space="PSUM")
                        for kc in range(3):
                            nc.tensor.matmul(ps[:pb, :], lhsT=hT[kc][:, ns],
                                             rhs=wsb[:, kc, :],
                                             start=(kc == 0), stop=False,
                                             skip_group_check=True)
                        nc.tensor.matmul(ps[:pb, :], lhsT=ones_b[:, ns],
                                         rhs=wsb[0:1, 3, :],
                                         start=False, stop=True,
                                         skip_group_check=True)
                        if name == "s":
                            xb = jp.tile([P, D], f32, tag="xb")
                            nc.sync.dma_start(xb[:pb, :], x_in[ns, :])
                            nc.vector.tensor_tensor(out=base[b][:pb, :],
                                                    in0=xb[:pb, :],
                                                    in1=ps[:pb, :], op=ALU.add)
                        elif name == "q":
                            if pb < P:
                                nc.vector.memset(q_sb[b][:, :], 0.0)
                            nc.vector.tensor_copy(q_sb[b][:pb, :], ps[:pb, :])
                        else:
                            ob = jp.tile([P, D], bf16, tag="proj_out")
                            nc.vector.tensor_copy(ob[:pb, :], ps[:pb, :])
                            if name == "k":
                                nc.sync.dma_start(kv_part[ns, 0:D], ob[:pb, :])
                            else:
                                nc.sync.dma_start(kv_part[ns, D:2 * D],
                                                  ob[:pb, :])

            hp_cm.__exit__(None, None, None)

            # ---- AllGather k,v ----
            if PH >= 3:
                if NOCC:
                    for cc in range(C):
                        nc.sync.dma_start(
                            kv_full[cc * NL:(cc + 1) * NL, :],
                            kv_part[:, :])
                else:
                    nc.gpsimd.collective_compute(
                        "AllGather", ALU.bypass, replica_groups=rg,
                        ins=[kv_part.ap().opt()], outs=[kv_full.ap().opt()])

            # ---- attention over edge tiles ----
            kidx_sb = cp.tile([P, T], i32, tag="kidx")
            nc.sync.dma_start(kidx_sb[:], kidx_in[:, :])

            nsup = T // TF if PH >= 4 else 0
            with tc.tile_pool(name="att", bufs=3) as ap_, \
                 tc.tile_pool(name="attS", bufs=2 * TF + 2) as sp2, \
                 tc.tile_pool(name="attps", bufs=2, space="PSUM") as pps, \
                 tc.tile_pool(name="aggps", bufs=1, space="PSUM") as agp, \
                 tc.tile_pool(name="qps", bufs=1, space="PSUM") as qpp, \
                 tc.tile_pool(name="fin", bufs=2) as fp:
                agg_ps = {}
                for sg in range(nsup):
                    ps_e = pps.tile([P, TF, 512], f32, tag="ps_e", space="PSUM")
                    kvsrc = ap_.tile([P, TF, 2 * D], bf16, tag="kvsrc")
                    ke = ap_.tile([P, TF, D], bf16, tag="ke")
                    prod = ap_.tile([P, TF, D], bf16, tag="prod")
                    rhs = ap_.tile([P, TF, D + H], bf16, tag="rhs")
                    lg = ap_.tile([P, TF, H], f32, tag="lg")
                    S_tiles = []
                    for j in range(TF):
                        t = TF * sg + j
                        ea_t = sp2.tile([P, 5, P], bf16, tag="ea")
                        nc.sync.dma_start(ea_t[:], eaS_in[t, :, :, :])
                        S_tiles.append(ea_t)
                        nc.gpsimd.indirect_dma_start(
                            out=kvsrc[:, j, :], out_offset=None,
                            in_=kv_full[:, :],
                            in_offset=bass.IndirectOffsetOnAxis(
                                ap=kidx_sb[:, t:t + 1], axis=0))
                        for kc in range(3):
                            nc.tensor.matmul(ps_e[:, j, 0:D],
                                             lhsT=ea_t[:, kc, :],
                                             rhs=we_sb[:, kc, :],
                                             start=(kc == 0), stop=False,
                                             skip_group_check=True)
                    # ke = k[src] + e   (fused over TF tiles)
                    nc.vector.tensor_tensor(out=ke[:, :, :],
                                            in0=kvsrc[:, :, 0:D],
                                            in1=ps_e[:, :, 0:D], op=ALU.add)
                    # psum += v[src]  ->  ve ; qdst = S^T @ q_block ; prod
                    for j in range(TF):
                        t = TF * sg + j
                        b = int(tile_block[t])
                        nc.tensor.matmul(ps_e[:, j, 0:D], lhsT=idn_b[:],
                                         rhs=kvsrc[:, j, D:2 * D], start=False,
                                         stop=True, skip_group_check=True)
                        q_ps = qpp.tile([P, 512], f32, tag="q_ps",
                                        name=f"qps{t}", space="PSUM")
                        nc.tensor.matmul(q_ps[:, 0:D],
                                         lhsT=S_tiles[j][:, 4, :],
                                         rhs=q_sb[b][:, :], start=True,
                                         stop=True, skip_group_check=True)
                        nc.vector.tensor_tensor(out=prod[:, j, :],
                                                in0=q_ps[:, 0:D],
                                                in1=ke[:, j, :], op=ALU.mult)
                    nc.vector.tensor_reduce(
                        out=lg[:, :, :],
                        in_=prod[:].rearrange("p t (h d) -> p t h d", h=H),
                        axis=mybir.AxisListType.X, op=ALU.add)
                    nc.scalar.activation(rhs[:, :, D:D + H], lg[:, :, :], AF.Exp,
                                         scale=float(SCALE), bias=zc[:, 0:1])
                    # wve = ve * w  (broadcast w over head dim)
                    nc.vector.tensor_tensor(
                        out=rhs[:, :, 0:D].rearrange("p t (h d) -> p t h d", h=H),
                        in0=ps_e[:, :, 0:D].rearrange("p t (h d) -> p t h d", h=H),
                        in1=rhs[:, :, D:D + H, None].to_broadcast(
                            [P, TF, H, DHEAD]),
                        op=ALU.mult)
                    # aggregate into per-block PSUM accumulators
                    for j in range(TF):
                        t = TF * sg + j
                        b = int(tile_block[t])
                        first = (t == blk_tile_start[b])
                        last = (t == blk_tile_start[b] + K[b] - 1)
                        if first:
                            agg_ps[b] = agp.tile([P, D + H], f32, tag="agg",
                                                 name=f"agg{b}", space="PSUM")
                        nc.tensor.matmul(agg_ps[b][:, :],
                                         lhsT=S_tiles[j][:, 3, :],
                                         rhs=rhs[:, j, :], start=first,
                                         stop=last, skip_group_check=True)
                        if last:
                            # finalize block: attn = agg/denom; x2 = base + attn
                            pb = _block_pb(b)
                            ag = agg_ps.pop(b)
                            if KDBG:
                                dbs = fp.tile([P, D + H], f32, tag="dbs")
                                nc.vector.tensor_copy(dbs[:], ag[:, :])
                                nc.sync.dma_start(
                                    dbg_dram[b * P:(b + 1) * P, :], dbs[:])
                            dn = fp.tile([P, H], f32, tag="dn")
                            nc.vector.tensor_scalar_max(dn[:], ag[:, D:D + H],
                                                        1e-30)
                            rd = fp.tile([P, H], f32, tag="rd")
                            nc.vector.reciprocal(rd[:], dn[:])
                            at = fp.tile([P, D], f32, tag="at")
                            for h in range(H):
                                nc.vector.tensor_scalar_mul(
                                    at[:pb, h * DHEAD:(h + 1) * DHEAD],
                                    ag[:pb, h * DHEAD:(h + 1) * DHEAD],
                                    rd[:pb, h:h + 1])
                            nc.vector.tensor_tensor(out=base[b][:pb, :],
                                                    in0=base[b][:pb, :],
                                                    in1=at[:pb, :], op=ALU.add)

            if PH < 5:
                with tc.tile_pool(name="dump", bufs=2) as dp:
                    for b in range(NBLK):
                        pb = _block_pb(b)
                        ns = slice(b * P, b * P + pb)
                        nc.sync.dma_start(out_dram[ns, :], base[b][:pb, :])
            if PH >= 5:
                # ---- transpose x2 (feature-major, bf16) + BN2 stats ----
                xp_cm = tc.tile_pool(name="x2tp", bufs=1)
                xp = xp_cm.__enter__()
                x2T = [xp.tile([P, NL], bf16, tag=f"x2T{c}", name=f"x2T{c}") for c in range(3)]
                with tc.tile_pool(name="tp", bufs=3, space="PSUM") as tpp:
                    for b in range(NBLK):
                        pb = _block_pb(b)
                        ns = slice(b * P, b * P + pb)
                        for dc in range(3):
                            tp_ps = tpp.tile([P, P], f32, tag="tp_ps", space="PSUM")
                            nc.tensor.transpose(
                                out=tp_ps[:, :pb],
                                in_=base[b][:pb, dc * P:(dc + 1) * P],
                                identity=idn_f[:pb, :pb])
                            nc.vector.tensor_copy(x2T[dc][:, ns], tp_ps[:, :pb])

                bn2_sb = cp.tile([P, 6], f32, tag="bn2sb")
                for c in range(3):
                    nc.vector.tensor_reduce(
                        out=bn2_sb[:, 2 * c:2 * c + 1], in_=x2T[c][:],
                        axis=mybir.AxisListType.X, op=ALU.add)
                    sq2 = xp.tile([P, NL], f32, tag="sq_bn2", name="sq2", bufs=2)
                    nc.vector.tensor_tensor(out=sq2[:], in0=x2T[c][:],
                                            in1=x2T[c][:], op=ALU.mult)
                    nc.vector.tensor_reduce(
                        out=bn2_sb[:, 2 * c + 1:2 * c + 2], in_=sq2[:],
                        axis=mybir.AxisListType.X, op=ALU.add)
                nc.sync.dma_start(bn2_i[:, :], bn2_sb[:])
                if NOCC:
                    nc.sync.dma_start(bn2_o[:, :], bn2_i[:, :])
                else:
                    nc.gpsimd.collective_compute(
                        "AllReduce", ALU.add, replica_groups=rg,
                        ins=[bn2_i.ap().opt()], outs=[bn2_o.ap().opt()])
                st2 = cp.tile([P, 6], f32, tag="st2")
                nc.sync.dma_start(st2[:], bn2_o[:, :])
                sc2, bi2 = bn_affine(st2, 2, 3, "sc2", "bi2")

                h2T = [pp.tile([P, NL], bf16, tag=f"h2T{c}", name=f"h2T{c}") for c in range(3)]
                for c in range(3):
                    nc.scalar.activation(h2T[c][:], x2T[c][:], AF.Identity,
                                         scale=sc2[:, c:c + 1], bias=bi2[:, c:c + 1])
                xp_cm.__exit__(None, None, None)
                if PH < 6:
                    with tc.tile_pool(name="dump", bufs=2) as dp:
                        for b in range(NBLK):
                            pb = _block_pb(b)
                            ns = slice(b * P, b * P + pb)
                            nc.sync.dma_start(out_dram[ns, :], base[b][:pb, :])
                if PH >= 6:
                    # ---- MLP (transposed) + residual + output ----
                    with tc.tile_pool(name="mlp", bufs=3) as mp, \
                         tc.tile_pool(name="mlpps", bufs=2, space="PSUM") as mpp, \
                         tc.tile_pool(name="mlpps2", bufs=2, space="PSUM") as mpp2:
                        for b in range(NBLK):
                            pb = _block_pb(b)
                            ns = slice(b * P, b * P + pb)
                            gT = []
                            for oc in range(H):
                                m1 = mpp.tile([P, P], f32, tag="m1", space="PSUM")
                                for kc in range(3):
                                    nc.tensor.matmul(
                                        m1[:, :pb],
                                        lhsT=w1_sb[:, kc, oc * P:(oc + 1) * P],
                                        rhs=h2T[kc][:, ns], start=(kc == 0),
                                        stop=(kc == 2), skip_group_check=True)
                                g_t = mp.tile([P, P], bf16, tag=f"gT{oc}")
                                nc.scalar.activation(g_t[:, :pb], m1[:, :pb], AF.Gelu,
                                                     bias=bm1_sb[:, oc:oc + 1])
                                gT.append(g_t)
                            outsb = mp.tile([P, D], f32, tag="outsb")
                            for dc in range(3):
                                m2 = mpp.tile([P, P], f32, tag="m2", space="PSUM")
                                for oc in range(H):
                                    nc.tensor.matmul(
                                        m2[:, :pb],
                                        lhsT=w2_sb[:, oc, dc * P:(dc + 1) * P],
                                        rhs=gT[oc][:, :pb], start=(oc == 0), stop=False,
                                        skip_group_check=True)
                                nc.tensor.matmul(m2[:, :pb],
                                                 lhsT=bm2_sb[0:1, dc * P:(dc + 1) * P],
                                                 rhs=ones_b[:, ns], start=False,
                                                 stop=True, skip_group_check=True)
                                m2sb = mp.tile([P, P], bf16, tag="m2sb")
                                nc.vector.tensor_copy(m2sb[:, :pb], m2[:, :pb])
                                m2tp = mpp2.tile([P, P], bf16, tag="m2tp", space="PSUM")
                                nc.tensor.transpose(out=m2tp[:pb, :], in_=m2sb[:, :pb],
                                                    identity=idn_b[:])
                                nc.vector.tensor_tensor(
                                    out=outsb[:pb, dc * P:(dc + 1) * P],
                                    in0=base[b][:pb, dc * P:(dc + 1) * P],
                                    in1=m2tp[:pb, :], op=ALU.add)
                            nc.sync.dma_start(out_dram[ns, :], outsb[:pb, :])
    nc.compile()
    return nc


_CACHE = {}


def kernel(x, edge_index, edge_attr, g1, b1, Wq, bq, Wk, bk, Wv, bv, We,
           Wskip, bskip, g2, b2, W1, bm1, W2, bm2):
    weights = (np.asarray(Wq, np.float32), np.asarray(bq, np.float32),
               np.asarray(Wk, np.float32), np.asarray(bk, np.float32),
               np.asarray(Wv, np.float32), np.asarray(bv, np.float32),
               np.asarray(We, np.float32),
               np.asarray(Wskip, np.float32), np.asarray(bskip, np.float32),
               np.asarray(g1, np.float32), np.asarray(b1, np.float32),
               np.asarray(g2, np.float32), np.asarray(b2, np.float32),
               np.asarray(W1, np.float32), np.asarray(bm1, np.float32),
               np.asarray(W2, np.float32), np.asarray(bm2, np.float32))
    in_maps, K, T = _prep_host(x, edge_index, edge_attr, weights)
    key = tuple(K)
    if key not in _CACHE:
        _CACHE[key] = _build(K, T)
    nc = _CACHE[key]
    res = run_bass_kernel_spmd(nc, in_maps, core_ids=list(range(C)))
    out = np.concatenate([res.results[c]["out"] for c in range(C)], axis=0)
    return out.astype(np.float32)


if __name__ == "__main__":
    import reference
    inputs = {k: np.asarray(v) for k, v in reference.setup_inputs().items()}
    got = kernel(**inputs)
    exp = np.asarray(reference.reference(**inputs))
    num = np.linalg.norm((got - exp).astype(np.float64))
    den = np.linalg.norm(exp.astype(np.float64))
    print("Relative error:", num / den)



# revision 10
# speedup vs baseline: 1.2901x; 1.2901x over previous
"""Trainium2 Bass kernel for nn_Block_71665824301263 (GNN message passing block).

Computation (see reference): BatchNorm -> TransformerConv-style edge attention
(6 heads, edge features added to K and V, segment softmax over incoming edges)
-> skip + residual -> BatchNorm -> MLP (gelu) -> residual.

Distribution over 8 NeuronCores:
- nodes sharded 2500/core; incoming edges partitioned by dst and sorted by dst
- weights replicated
- k/v projections AllGather'ed (bf16) so every core can gather src rows
- BatchNorm statistics AllReduce'd (2x [384] sums per BN)

Schedule (v2):
- BN stats via PE ones-matmuls on node-major x blocks (column sums into PSUM)
  instead of DVE free-dim reductions over feature-major transposes.
- Edge We-matmuls for the first supers are emitted before the projections so
  the PE has work during the BN1 AllReduce window.
- k/v projections run first and kick the kv AllGather; q/skip projections and
  eaS prefetch overlap the collective.
- Attention loop: supers of TF=2 tiles; PSUM fully double-buffered
  (We 2x2 banks, q-broadcast 2 banks, agg 2 banks); ACT evacuates PSUM to
  bf16 SBUF so every bulk DVE op runs in 2x bf16 mode.
- MLP batched over 4 node blocks (N=512 matmuls).
"""

import os
import numpy as np
import ml_dtypes

import concourse.bass as bass
import concourse.bacc as bacc
import concourse.tile as tile
import concourse.mybir as mybir
from concourse.bass_utils import run_bass_kernel_spmd
from concourse.masks import make_identity

C = 8            # cores
N = 20000        # nodes
NL = N // C      # nodes per core
D = 384
H = 6
DHEAD = 64
DH = 2 * D       # mlp hidden
P = 128
NBLK = (NL + P - 1) // P      # 20 node blocks per core (last has 68)
TF = 2                        # tiles per super (PSUM double-buffer friendly)
NPRE = 8                      # supers whose We-matmuls are hoisted pre-proj
SCALE = 1.0 / np.sqrt(np.float32(DHEAD))
EPS = 1e-5

f32 = mybir.dt.float32
bf16 = mybir.dt.bfloat16
i32 = mybir.dt.int32
BF = ml_dtypes.bfloat16
AF = mybir.ActivationFunctionType
ALU = mybir.AluOpType


def _block_pb(b):
    return min(P, NL - b * P)


def _prep_host(x, edge_index, edge_attr, weights):
    """Shard + pad edges, build per-core input maps. Returns (in_maps, K, T)."""
    src = np.asarray(edge_index[0]).astype(np.int64)
    dst = np.asarray(edge_index[1]).astype(np.int64)
    x = np.asarray(x, dtype=np.float32)
    edge_attr = np.asarray(edge_attr, dtype=np.float32)

    cores = []
    cnt = np.zeros((C, NBLK), np.int64)
    for c in range(C):
        sel = (dst >= c * NL) & (dst < (c + 1) * NL)
        eids = np.nonzero(sel)[0]
        d_loc = (dst[eids] - c * NL).astype(np.int64)
        order = np.argsort(d_loc, kind="stable")
        eids = eids[order]
        d_loc = d_loc[order]
        s_glob = src[eids]
        blk = d_loc // P
        cnt[c] = np.bincount(blk, minlength=NBLK)
        cores.append((eids, d_loc, s_glob, blk))

    K = [max(1, int(-(-cnt[:, b].max() // P))) for b in range(NBLK)]
    T = sum(K)
    pad = (-T) % TF
    K[NBLK - 1] += pad
    T += pad
    tile_block = np.repeat(np.arange(NBLK), K)          # block id per tile
    blk_tile_start = np.concatenate([[0], np.cumsum(K)])[:NBLK]
    blk_edge_start = blk_tile_start * P

    # replicated weight tensors
    def chunks(w, nk):
        return np.stack([w[i * P:(i + 1) * P] for i in range(nk)]).astype(BF)

    (Wq, bq, Wk, bk, Wv, bv, We, Wskip, bskip,
     g1, b1, g2, b2, W1, bm1, W2, bm2) = weights

    def aug(w, b):
        a = np.zeros((4, P, w.shape[1]), np.float32)
        a[:3] = np.stack([w[i * P:(i + 1) * P] for i in range(3)])
        a[3, 0] = b
        return a.astype(BF)

    shared = {
        "Wq": aug(Wq, bq), "Wk": aug(Wk, bk), "Wv": aug(Wv, bv),
        "Wsk": aug(Wskip, bskip),
        "We": chunks(We, 3),
        "W1": aug(W1, bm1)[:3],                 # bias separately (bm1T)
        "bm1T": np.asarray(bm1, np.float32).reshape(H, P).T.copy(),
        "W2": chunks(W2, 6),
        "bm2": np.asarray(bm2, np.float32).reshape(1, D).astype(BF),
        "gb": np.stack([np.asarray(v, np.float32).reshape(3, P)
                        for v in (g1, b1, g2, b2)], axis=-1),  # [3, P, 4]
    }

    in_maps = []
    for c in range(C):
        eids, d_loc, s_glob, blk = cores[c]
        starts = np.searchsorted(blk, np.arange(NBLK))
        rank = np.arange(len(blk)) - starts[blk]
        pos = blk_edge_start[blk] + rank

        src_pad = np.zeros(T * P, np.int64)
        dst_pad = np.zeros(T * P, np.int64)
        valid = np.zeros(T * P, bool)
        src_pad[pos] = s_glob
        dst_pad[pos] = d_loc
        valid[pos] = True

        ea_pad = np.zeros((T * P, D), np.float32)
        ea_pad[pos] = edge_attr[eids]
        eaT = ea_pad.astype(BF).reshape(T, P, 3, P).transpose(0, 3, 2, 1)

        S = np.zeros((T * P, P), np.float32)
        tb = np.repeat(tile_block, P)
        S[np.nonzero(valid)[0], (dst_pad - tb * P)[valid]] = 1.0
        S = S.astype(BF).reshape(T, P, P)

        # combined [T, P, 5, P]: chunks 0-2 = eaT (partition=feature),
        # chunk 3 = S (partition=edge), chunk 4 = S^T (partition=node)
        ST = S.transpose(0, 2, 1)
        eaS = np.concatenate([eaT, S[:, :, None, :], ST[:, :, None, :]], axis=2)
        eaS = np.ascontiguousarray(eaS)

        kidx = np.where(valid, src_pad, 0)
        kidxT = np.ascontiguousarray(kidx.reshape(T, P).T).astype(np.int32)

        x_loc = np.ascontiguousarray(x[c * NL:(c + 1) * NL])
        xT_loc = np.ascontiguousarray(x_loc.T)

        m = {"x_loc": x_loc, "xT_loc": xT_loc, "eaS": eaS, "kidx": kidxT}
        m.update(shared)
        in_maps.append(m)
    return in_maps, K, T


def _build(K, T, zero_bias=False):
    NOCC = int(os.environ.get("KNOCC", "0"))
    LAG = 2                                    # agg matmul software pipeline
    nc = bacc.Bacc("TRN2", target_bir_lowering=False, debug=False,
                   enable_asserts=False, num_devices=C)
    tile_block = np.repeat(np.arange(NBLK), K)
    blk_tile_start = np.concatenate([[0], np.cumsum(K)])[:NBLK]
    NSUP = T // TF

    # ------------- I/O -------------
    x_in = nc.dram_tensor("x_loc", [NL, D], f32, kind="ExternalInput")
    xT_in = nc.dram_tensor("xT_loc", [D, NL], f32, kind="ExternalInput")
    eaS_in = nc.dram_tensor("eaS", [T, P, 5, P], bf16, kind="ExternalInput")
    kidx_in = nc.dram_tensor("kidx", [P, T], i32, kind="ExternalInput")
    wq_in = nc.dram_tensor("Wq", [4, P, D], bf16, kind="ExternalInput")
    wk_in = nc.dram_tensor("Wk", [4, P, D], bf16, kind="ExternalInput")
    wv_in = nc.dram_tensor("Wv", [4, P, D], bf16, kind="ExternalInput")
    wsk_in = nc.dram_tensor("Wsk", [4, P, D], bf16, kind="ExternalInput")
    we_in = nc.dram_tensor("We", [3, P, D], bf16, kind="ExternalInput")
    w1_in = nc.dram_tensor("W1", [3, P, DH], bf16, kind="ExternalInput")
    bm1_in = nc.dram_tensor("bm1T", [P, H], f32, kind="ExternalInput")
    w2_in = nc.dram_tensor("W2", [6, P, D], bf16, kind="ExternalInput")
    bm2_in = nc.dram_tensor("bm2", [1, D], bf16, kind="ExternalInput")
    gb_in = nc.dram_tensor("gb", [3, P, 4], f32, kind="ExternalInput")
    out_dram = nc.dram_tensor("out", [NL, D], f32, kind="ExternalOutput")

    # ------------- internal DRAM -------------
    kv_part = nc.dram_tensor("kv_part", [NL, 2 * D], bf16)
    kv_full = nc.dram_tensor("kv_full", [C * NL, 2 * D], bf16,
                             addr_space="Shared")
    bn1_i = nc.dram_tensor("bn1_i", [P, 6], f32)
    bn1_o = nc.dram_tensor("bn1_o", [P, 6], f32, addr_space="Shared")
    bn2_i = nc.dram_tensor("bn2_i", [P, 6], f32)
    bn2_o = nc.dram_tensor("bn2_o", [P, 6], f32, addr_space="Shared")

    rg = [list(range(C))]

    with tile.TileContext(nc) as tc:
        with tc.tile_pool(name="const", bufs=1) as cp, \
             tc.tile_pool(name="persist", bufs=1) as pp, \
             tc.tile_pool(name="small", bufs=2) as sp:
            # ---- constants / weights ----
            idn_f = cp.tile([P, P], f32, tag="idn_f")
            make_identity(nc, idn_f[:])
            idn_b = cp.tile([P, P], bf16, tag="idn_b")
            make_identity(nc, idn_b[:])
            ones_b = cp.tile([1, NL], bf16, tag="ones_b")
            nc.vector.memset(ones_b[:], 1.0)
            ones_cf = cp.tile([P, 1], f32, tag="ones_cf")
            nc.vector.memset(ones_cf[:], 1.0)
            ones_cb = cp.tile([P, 1], bf16, tag="ones_cb")
            nc.vector.memset(ones_cb[:], 1.0)
            epst = cp.tile([P, 1], f32, tag="epst")
            nc.vector.memset(epst[:], EPS)
            wq_sb = cp.tile([P, 4, D], bf16, tag="wq")
            nc.sync.dma_start(wq_sb[:], wq_in.ap().rearrange("c p d -> p c d"))
            wk_sb = cp.tile([P, 4, D], bf16, tag="wk")
            nc.sync.dma_start(wk_sb[:], wk_in.ap().rearrange("c p d -> p c d"))
            wv_sb = cp.tile([P, 4, D], bf16, tag="wv")
            nc.sync.dma_start(wv_sb[:], wv_in.ap().rearrange("c p d -> p c d"))
            wsk_sb = cp.tile([P, 4, D], bf16, tag="wsk")
            nc.sync.dma_start(wsk_sb[:], wsk_in.ap().rearrange("c p d -> p c d"))
            we_sb = cp.tile([P, 3, D], bf16, tag="we")
            nc.sync.dma_start(we_sb[:], we_in.ap().rearrange("c p d -> p c d"))
            w1_sb = cp.tile([P, 3, DH], bf16, tag="w1")
            nc.sync.dma_start(w1_sb[:], w1_in.ap().rearrange("c p d -> p c d"))
            bm1_sb = cp.tile([P, H], f32, tag="bm1")
            nc.sync.dma_start(bm1_sb[:], bm1_in[:, :])
            w2_sb = cp.tile([P, 6, D], bf16, tag="w2")
            nc.sync.dma_start(w2_sb[:], w2_in.ap().rearrange("c p d -> p c d"))
            bm2_sb = cp.tile([1, D], bf16, tag="bm2")
            nc.sync.dma_start(bm2_sb[:], bm2_in[:, :])
            gb_sb = cp.tile([P, 3, 4], f32, tag="gb")
            nc.sync.dma_start(gb_sb[:], gb_in.ap().rearrange("c p j -> p c j"))
            kidx_sb = cp.tile([P, T], i32, tag="kidx")
            nc.sync.dma_start(kidx_sb[:], kidx_in[:, :])

            # ---- BN1 stats via PE column sums over node-major x blocks ----
            base = [pp.tile([P, D], f32, tag=f"base{b}", name=f"base{b}")
                    for b in range(NBLK)]

            def pe_stats(src_tiles, stats_sb, sqtag):
                """Column sums + sums of squares of per-block node-major f32
                tiles -> stats_sb [P, 6] (chunk-major: sum,sumsq per chunk)."""
                with tc.tile_pool(name=sqtag, bufs=3) as qp_, \
                     tc.tile_pool(name=sqtag + "ps", bufs=1,
                                  space="PSUM") as qps:
                    s_ps = qps.tile([1, 512], f32, tag="s_ps", space="PSUM")
                    q_ps = qps.tile([1, 512], f32, tag="q_ps", space="PSUM")
                    for b in range(NBLK):
                        pb = _block_pb(b)
                        sq = qp_.tile([P, D], bf16, tag="sq")
                        nc.scalar.activation(sq[:pb, :], src_tiles[b][:pb, :],
                                             AF.Square)
                        nc.tensor.matmul(s_ps[0:1, 0:D], lhsT=ones_cf[:pb, :],
                                         rhs=src_tiles[b][:pb, :],
                                         start=(b == 0), stop=(b == NBLK - 1),
                                         skip_group_check=True)
                        nc.tensor.matmul(q_ps[0:1, 0:D], lhsT=ones_cb[:pb, :],
                                         rhs=sq[:pb, :],
                                         start=(b == 0), stop=(b == NBLK - 1),
                                         skip_group_check=True)
                    row = qp_.tile([1, 2, D], f32, tag="strow")
                    nc.scalar.activation(row[0:1, 0, :], s_ps[0:1, 0:D], AF.Copy)
                    nc.scalar.activation(row[0:1, 1, :], q_ps[0:1, 0:D], AF.Copy)
                    # transpose [1, 384]x2 -> [P, 6] (chunk c: sum, sumsq)
                    tp_ps = qps.tile([P, 6], f32, tag="tp_ps", space="PSUM")
                    for cch in range(3):
                        for s_ in range(2):
                            nc.tensor.transpose(
                                out=tp_ps[:, 2 * cch + s_:2 * cch + s_ + 1],
                                in_=row[0:1, s_, cch * P:(cch + 1) * P],
                                identity=idn_f[0:1, 0:1])
                    nc.vector.tensor_copy(stats_sb[:], tp_ps[:, :])

            bn1_sb = cp.tile([P, 6], f32, tag="bn1sb")
            with tc.tile_pool(name="xload", bufs=4) as xlp:
                for b in range(NBLK):
                    pb = _block_pb(b)
                    nc.sync.dma_start(base[b][:pb, :],
                                      x_in[b * P:b * P + pb, :])
            pe_stats(base, bn1_sb, "bn1q")
            nc.sync.dma_start(bn1_i[:, :], bn1_sb[:])
            if NOCC:
                nc.sync.dma_start(bn1_o[:, :], bn1_i[:, :])
            else:
                nc.gpsimd.collective_compute(
                    "AllReduce", ALU.add, replica_groups=rg,
                    ins=[bn1_i.ap().opt()], outs=[bn1_o.ap().opt()])

            # ---- attention pools (created early: frontA prewarm uses them) ----
            att_ea_cm = tc.tile_pool(name="att_ea", bufs=16)
            eap = att_ea_cm.__enter__()
            att_e_cm = tc.tile_pool(name="att_e", bufs=16)
            aep = att_e_cm.__enter__()
            att_eps_cm = tc.tile_pool(name="att_eps", bufs=2, space="PSUM")
            pps = att_eps_cm.__enter__()

            ea_tiles = {}
            e_tiles = {}

            def frontA(sg):
                """Edge-feature work with no BN/AllGather dependency:
                eaS DMA, We matmuls, ACT evac of e to bf16 SBUF."""
                t0 = TF * sg
                ea_t = eap.tile([P, TF, 5, P], bf16, tag="ea")
                nc.sync.dma_start(
                    ea_t[:], eaS_in.ap()[t0:t0 + TF]
                    .rearrange("t p c x -> p t c x"))
                e_ps = pps.tile([P, TF, 512], f32, tag="e_ps", space="PSUM")
                for j in range(TF):
                    for kc in range(3):
                        nc.tensor.matmul(e_ps[:, j, 0:D],
                                         lhsT=ea_t[:, j, kc, :],
                                         rhs=we_sb[:, kc, :],
                                         start=(kc == 0), stop=(kc == 2),
                                         skip_group_check=True)
                e_sb = aep.tile([P, TF, D], bf16, tag="e_sb")
                nc.scalar.activation(e_sb[:], e_ps[:, :, 0:D], AF.Copy)
                ea_tiles[sg] = ea_t
                e_tiles[sg] = e_sb

            # pre-warm during the BN1 AllReduce window
            for sg in range(min(NPRE, NSUP)):
                frontA(sg)

            # ---- BN1 affine + hT ----
            st1 = cp.tile([P, 6], f32, tag="st1")
            nc.sync.dma_start(st1[:], bn1_o[:, :])

            def bn_affine(st, gcol, bcol, scn, bin_):
                sc_t = cp.tile([P, 3], f32, tag=scn)
                bi_t = cp.tile([P, 3], f32, tag=bin_)
                for c in range(3):
                    mean = sp.tile([P, 1], f32, tag="bn_mean")
                    nc.vector.tensor_scalar_mul(mean[:], st[:, 2 * c:2 * c + 1],
                                                1.0 / N)
                    var = sp.tile([P, 1], f32, tag="bn_var")
                    nc.vector.tensor_scalar_mul(var[:], st[:, 2 * c + 1:2 * c + 2],
                                                1.0 / N)
                    msq = sp.tile([P, 1], f32, tag="bn_msq")
                    nc.vector.tensor_tensor(out=msq[:], in0=mean[:], in1=mean[:],
                                            op=ALU.mult)
                    nc.vector.tensor_tensor(out=var[:], in0=var[:], in1=msq[:],
                                            op=ALU.subtract)
                    std = sp.tile([P, 1], f32, tag="bn_std")
                    nc.scalar.activation(std[:], var[:], AF.Sqrt, bias=epst[:, 0:1])
                    rstd = sp.tile([P, 1], f32, tag="bn_rstd")
                    nc.vector.reciprocal(rstd[:], std[:])
                    nc.vector.tensor_tensor(out=sc_t[:, c:c + 1], in0=rstd[:],
                                            in1=gb_sb[:, c, gcol:gcol + 1],
                                            op=ALU.mult)
                    ms = sp.tile([P, 1], f32, tag="bn_ms")
                    nc.vector.tensor_tensor(out=ms[:], in0=mean[:],
                                            in1=sc_t[:, c:c + 1], op=ALU.mult)
                    nc.vector.tensor_tensor(out=bi_t[:, c:c + 1],
                                            in0=gb_sb[:, c, bcol:bcol + 1],
                                            in1=ms[:], op=ALU.subtract)
                return sc_t, bi_t

            sc1, bi1 = bn_affine(st1, 0, 1, "sc1", "bi1")

            hp_cm = tc.tile_pool(name="bnh", bufs=1)
            hp = hp_cm.__enter__()
            hT = [hp.tile([P, NL], bf16, tag=f"hT{c}", name=f"hT{c}")
                  for c in range(3)]
            with tc.tile_pool(name="xTl", bufs=2) as xtp:
                for c in range(3):
                    xT_t = xtp.tile([P, NL], f32, tag="xT")
                    nc.sync.dma_start(xT_t[:], xT_in[c * P:(c + 1) * P, :])
                    nc.vector.tensor_scalar(out=hT[c][:], in0=xT_t[:],
                                            scalar1=sc1[:, c:c + 1],
                                            scalar2=bi1[:, c:c + 1],
                                            op0=ALU.mult, op1=ALU.add)

            # ---- k,v projections -> AllGather ----
            q_sb = [pp.tile([P, D], bf16, tag=f"q{b}", name=f"q{b}")
                    for b in range(NBLK)]
            with tc.tile_pool(name="proj", bufs=3) as jp, \
                 tc.tile_pool(name="projps", bufs=3, space="PSUM") as jpp:
                def proj_mm(ps, wsb, ns, pb):
                    for kc in range(3):
                        nc.tensor.matmul(ps[:pb, :], lhsT=hT[kc][:, ns],
                                         rhs=wsb[:, kc, :],
                                         start=(kc == 0),
                                         stop=(kc == 2 and zero_bias),
                                         skip_group_check=True)
                    if not zero_bias:
                        nc.tensor.matmul(ps[:pb, :], lhsT=ones_b[:, ns],
                                         rhs=wsb[0:1, 3, :],
                                         start=False, stop=True,
                                         skip_group_check=True)

                for b in range(NBLK):
                    pb = _block_pb(b)
                    ns = slice(b * P, b * P + pb)
                    kv_t = jp.tile([P, 2 * D], bf16, tag="kv_t")
                    for wsb, off in ((wk_sb, 0), (wv_sb, D)):
                        ps = jpp.tile([P, D], f32, tag="proj_ps", space="PSUM")
                        proj_mm(ps, wsb, ns, pb)
                        nc.vector.tensor_copy(kv_t[:pb, off:off + D],
                                              ps[:pb, :])
                    nc.sync.dma_start(kv_part[ns, :], kv_t[:pb, :])

                if NOCC:
                    for cc in range(C):
                        nc.sync.dma_start(
                            kv_full[cc * NL:(cc + 1) * NL, :], kv_part[:, :])
                else:
                    nc.gpsimd.collective_compute(
                        "AllGather", ALU.bypass, replica_groups=rg,
                        ins=[kv_part.ap().opt()], outs=[kv_full.ap().opt()])

                # ---- q, skip projections (overlap the AllGather) ----
                for b in range(NBLK):
                    pb = _block_pb(b)
                    ns = slice(b * P, b * P + pb)
                    for wsb, name in ((wq_sb, "q"), (wsk_sb, "s")):
                        ps = jpp.tile([P, D], f32, tag="proj_ps", space="PSUM")
                        proj_mm(ps, wsb, ns, pb)
                        if name == "q":
                            if pb < P:
                                nc.vector.memset(q_sb[b][:, :], 0.0)
                            nc.vector.tensor_copy(q_sb[b][:pb, :], ps[:pb, :])
                        else:
                            nc.vector.tensor_tensor(out=base[b][:pb, :],
                                                    in0=base[b][:pb, :],
                                                    in1=ps[:pb, :], op=ALU.add)
            hp_cm.__exit__(None, None, None)

            # ---- attention over edge supers (TF tiles, agg lagged by LAG) ----
            with tc.tile_pool(name="att_kv", bufs=3) as kvp, \
                 tc.tile_pool(name="att_sb", bufs=3) as asb, \
                 tc.tile_pool(name="att_rhs", bufs=LAG + 2) as rsp, \
                 tc.tile_pool(name="att_qps", bufs=2, space="PSUM") as qpp, \
                 tc.tile_pool(name="att_agg", bufs=2, space="PSUM") as agp, \
                 tc.tile_pool(name="fin", bufs=2) as fp:
                agg_ps = {}
                rhs_tiles = {}

                def frontB(sg):
                    t0 = TF * sg
                    ea_t = ea_tiles[sg]
                    e_sb = e_tiles[sg]
                    kvsrc = kvp.tile([P, TF, 2 * D], bf16, tag="kvsrc")
                    for j in range(TF):
                        nc.gpsimd.indirect_dma_start(
                            out=kvsrc[:, j, :], out_offset=None,
                            in_=kv_full[:, :],
                            in_offset=bass.IndirectOffsetOnAxis(
                                ap=kidx_sb[:, t0 + j:t0 + j + 1], axis=0))

                    # q[dst] broadcast via S^T matmuls; ACT evacuates to bf16
                    qd_sb = asb.tile([P, TF, D], bf16, tag="qd_sb")
                    for j in range(TF):
                        b = int(tile_block[t0 + j])
                        q_ps = qpp.tile([P, 512], f32, tag="q_ps", space="PSUM")
                        nc.tensor.matmul(q_ps[:, 0:D],
                                         lhsT=ea_t[:, j, 4, :],
                                         rhs=q_sb[b][:, :], start=True,
                                         stop=True, skip_group_check=True)
                        nc.scalar.activation(qd_sb[:, j, :], q_ps[:, 0:D],
                                             AF.Copy)

                    # logits = reduce_h(qd * (k[src] + e)) ; w = exp(scale*lg)
                    ke = asb.tile([P, TF, D], bf16, tag="ke")
                    nc.vector.tensor_tensor(out=ke[:], in0=kvsrc[:, :, 0:D],
                                            in1=e_sb[:], op=ALU.add)
                    prod = asb.tile([P, TF, D], bf16, tag="prod")
                    nc.vector.tensor_tensor(out=prod[:], in0=qd_sb[:],
                                            in1=ke[:], op=ALU.mult)
                    lg = asb.tile([P, TF, H], f32, tag="lg")
                    nc.vector.tensor_reduce(
                        out=lg[:],
                        in_=prod[:].rearrange("p t (h d) -> p t h d", h=H),
                        axis=mybir.AxisListType.X, op=ALU.add)
                    rhs = rsp.tile([P, TF, D + H], bf16, tag="rhs")
                    nc.scalar.activation(rhs[:, :, D:D + H], lg[:], AF.Exp,
                                         scale=float(SCALE))
                    ve = asb.tile([P, TF, D], bf16, tag="ve")
                    nc.vector.tensor_tensor(out=ve[:], in0=kvsrc[:, :, D:2 * D],
                                            in1=e_sb[:], op=ALU.add)
                    nc.vector.tensor_tensor(
                        out=rhs[:, :, 0:D].rearrange("p t (h d) -> p t h d", h=H),
                        in0=ve[:].rearrange("p t (h d) -> p t h d", h=H),
                        in1=rhs[:, :, D:D + H, None].to_broadcast(
                            [P, TF, H, DHEAD]),
                        op=ALU.mult)
                    rhs_tiles[sg] = rhs

                def emit_agg(sg):
                    t0 = TF * sg
                    ea_t = ea_tiles.pop(sg)
                    e_tiles.pop(sg)
                    rhs = rhs_tiles.pop(sg)
                    for j in range(TF):
                        t = t0 + j
                        b = int(tile_block[t])
                        first = (t == blk_tile_start[b])
                        last = (t == blk_tile_start[b] + K[b] - 1)
                        if first:
                            agg_ps[b] = agp.tile([P, 512], f32, tag="agg",
                                                 name=f"agg{b}", space="PSUM")
                        nc.tensor.matmul(agg_ps[b][:, 0:D + H],
                                         lhsT=ea_t[:, j, 3, :],
                                         rhs=rhs[:, j, :], start=first,
                                         stop=last, skip_group_check=True)
                        if last:
                            pb = _block_pb(b)
                            ag = agg_ps.pop(b)
                            ag_sb = fp.tile([P, D + H], f32, tag="ag_sb")
                            nc.scalar.activation(ag_sb[:pb, :],
                                                 ag[:pb, 0:D + H], AF.Copy)
                            dn = fp.tile([P, H], f32, tag="dn")
                            nc.vector.tensor_scalar_max(dn[:pb, :],
                                                        ag_sb[:pb, D:D + H],
                                                        1e-30)
                            rd = fp.tile([P, H], f32, tag="rd")
                            nc.vector.reciprocal(rd[:pb, :], dn[:pb, :])
                            at = fp.tile([P, D], f32, tag="at")
                            nc.vector.tensor_tensor(
                                out=at[:pb, :].rearrange(
                                    "p (h d) -> p h d", h=H),
                                in0=ag_sb[:pb, 0:D].rearrange(
                                    "p (h d) -> p h d", h=H),
                                in1=rd[:pb, :, None].to_broadcast(
                                    [pb, H, DHEAD]),
                                op=ALU.mult)
                            nc.vector.tensor_tensor(out=base[b][:pb, :],
                                                    in0=base[b][:pb, :],
                                                    in1=at[:pb, :], op=ALU.add)

                for sg in range(NSUP + LAG):
                    if sg < NSUP:
                        if sg >= NPRE:
                            frontA(sg)
                        frontB(sg)
                    if sg >= LAG:
                        emit_agg(sg - LAG)

            att_eps_cm.__exit__(None, None, None)
            att_e_cm.__exit__(None, None, None)
            att_ea_cm.__exit__(None, None, None)

            # ---- BN2 stats (PE, node-major) + AllReduce ----
            bn2_sb = cp.tile([P, 6], f32, tag="bn2sb")
            pe_stats(base, bn2_sb, "bn2q")
            nc.sync.dma_start(bn2_i[:, :], bn2_sb[:])
            if NOCC:
                nc.sync.dma_start(bn2_o[:, :], bn2_i[:, :])
            else:
                nc.gpsimd.collective_compute(
                    "AllReduce", ALU.add, replica_groups=rg,
                    ins=[bn2_i.ap().opt()], outs=[bn2_o.ap().opt()])

            # ---- x2 transposes (overlap the AllReduce) ----
            xp_cm = tc.tile_pool(name="x2tp", bufs=1)
            xp = xp_cm.__enter__()
            x2T = [xp.tile([P, NL], bf16, tag=f"x2T{c}", name=f"x2T{c}")
                   for c in range(3)]
            with tc.tile_pool(name="tp", bufs=3, space="PSUM") as tpp:
                for b in range(NBLK):
                    pb = _block_pb(b)
                    ns = slice(b * P, b * P + pb)
                    for dc in range(3):
                        tp_ps = tpp.tile([P, P], f32, tag="tp_ps", space="PSUM")
                        nc.tensor.transpose(
                            out=tp_ps[:, :pb],
                            in_=base[b][:pb, dc * P:(dc + 1) * P],
                            identity=idn_f[:pb, :pb])
                        nc.scalar.activation(x2T[dc][:, ns], tp_ps[:, :pb],
                                             AF.Copy)

            st2 = cp.tile([P, 6], f32, tag="st2")
            nc.sync.dma_start(st2[:], bn2_o[:, :])
            sc2, bi2 = bn_affine(st2, 2, 3, "sc2", "bi2")

            h2T = [xp.tile([P, NL], bf16, tag=f"h2T{c}", name=f"h2T{c}")
                   for c in range(3)]
            for c in range(3):
                nc.vector.tensor_scalar(out=h2T[c][:], in0=x2T[c][:],
                                        scalar1=sc2[:, c:c + 1],
                                        scalar2=bi2[:, c:c + 1],
                                        op0=ALU.mult, op1=ALU.add)

            # ---- MLP batched over 4 node blocks (N=512) ----
            GB = 4
            with tc.tile_pool(name="mlp", bufs=2) as mp, \
                 tc.tile_pool(name="mlpps", bufs=3, space="PSUM") as mpp, \
                 tc.tile_pool(name="mlpps2", bufs=2, space="PSUM") as mpp2:
                for g in range(0, NBLK, GB):
                    blks = list(range(g, min(g + GB, NBLK)))
                    n0 = g * P
                    ng = sum(_block_pb(b) for b in blks)
                    gs = slice(n0, n0 + ng)
                    gT = []
                    for oc in range(H):
                        m1 = mpp.tile([P, 512], f32, tag="m1", space="PSUM")
                        for kc in range(3):
                            nc.tensor.matmul(
                                m1[:, :ng],
                                lhsT=w1_sb[:, kc, oc * P:(oc + 1) * P],
                                rhs=h2T[kc][:, gs], start=(kc == 0),
                                stop=(kc == 2), skip_group_check=True)
                        g_t = mp.tile([P, 512], bf16, tag=f"gT{oc}")
                        nc.scalar.activation(g_t[:, :ng], m1[:, :ng], AF.Gelu,
                                             bias=bm1_sb[:, oc:oc + 1])
                        gT.append(g_t)
                    for dc in range(3):
                        m2 = mpp.tile([P, 512], f32, tag="m2", space="PSUM")
                        for oc in range(H):
                            nc.tensor.matmul(
                                m2[:, :ng],
                                lhsT=w2_sb[:, oc, dc * P:(dc + 1) * P],
                                rhs=gT[oc][:, :ng], start=(oc == 0),
                                stop=(oc == H - 1 and zero_bias),
                                skip_group_check=True)
                        if not zero_bias:
                            nc.tensor.matmul(
                                m2[:, :ng],
                                lhsT=bm2_sb[0:1, dc * P:(dc + 1) * P],
                                rhs=ones_b[:, gs], start=False,
                                stop=True, skip_group_check=True)
                        m2sb = mp.tile([P, 512], bf16, tag="m2sb")
                        nc.vector.tensor_copy(m2sb[:, :ng], m2[:, :ng])
                        for bi_, b in enumerate(blks):
                            pb = _block_pb(b)
                            m2tp = mpp2.tile([P, P], bf16, tag="m2tp",
                                             space="PSUM")
                            nc.tensor.transpose(
                                out=m2tp[:pb, :],
                                in_=m2sb[:, bi_ * P:bi_ * P + pb],
                                identity=idn_b[:])
                            nc.vector.tensor_tensor(
                                out=base[b][:pb, dc * P:(dc + 1) * P],
                                in0=base[b][:pb, dc * P:(dc + 1) * P],
                                in1=m2tp[:pb, :], op=ALU.add)
                    for b in blks:
                        pb = _block_pb(b)
                        ns = slice(b * P, b * P + pb)
                        nc.sync.dma_start(out_dram[ns, :], base[b][:pb, :])
            xp_cm.__exit__(None, None, None)
    nc.compile()
    return nc


_CACHE = {}


def kernel(x, edge_index, edge_attr, g1, b1, Wq, bq, Wk, bk, Wv, bv, We,
           Wskip, bskip, g2, b2, W1, bm1, W2, bm2):
    weights = (np.asarray(Wq, np.float32), np.asarray(bq, np.float32),
               np.asarray(Wk, np.float32), np.asarray(bk, np.float32),
               np.asarray(Wv, np.float32), np.asarray(bv, np.float32),
               np.asarray(We, np.float32),
               np.asarray(Wskip, np.float32), np.asarray(bskip, np.float32),
               np.asarray(g1, np.float32), np.asarray(b1, np.float32),
               np.asarray(g2, np.float32), np.asarray(b2, np.float32),
               np.asarray(W1, np.float32), np.asarray(bm1, np.float32),
               np.asarray(W2, np.float32), np.asarray(bm2, np.float32))
    in_maps, K, T = _prep_host(x, edge_index, edge_attr, weights)
    zb = all(not np.any(np.asarray(v)) for v in (bq, bk, bv, bskip, bm2))
    key = (tuple(K), zb)
    if key not in _CACHE:
        _CACHE[key] = _build(K, T, zero_bias=zb)
    nc = _CACHE[key]
    res = run_bass_kernel_spmd(nc, in_maps, core_ids=list(range(C)))
    out = np.concatenate([res.results[c]["out"] for c in range(C)], axis=0)
    return out.astype(np.float32)


if __name__ == "__main__":
    import reference
    inputs = {k: np.asarray(v) for k, v in reference.setup_inputs().items()}
    got = kernel(**inputs)
    exp = np.asarray(reference.reference(**inputs))
    num = np.linalg.norm((got - exp).astype(np.float64))
    den = np.linalg.norm(exp.astype(np.float64))
    print("Relative error:", num / den)


# revision 22
# speedup vs baseline: 1.3679x; 1.0603x over previous
"""Trainium2 Bass kernel for nn_Block_71665824301263 (GNN message passing block).

Computation (see reference): BatchNorm -> TransformerConv-style edge attention
(6 heads, edge features added to K and V, segment softmax over incoming edges)
-> skip + residual -> BatchNorm -> MLP (gelu) -> residual.

Distribution over 8 NeuronCores:
- nodes sharded 2500/core; incoming edges partitioned by dst and sorted by dst
- weights replicated
- k/v projections AllGather'ed (bf16) so every core can gather src rows
- BatchNorm statistics AllReduce'd (2x [384] sums per BN)

Schedule (v2):
- BN stats via PE ones-matmuls on node-major x blocks (column sums into PSUM)
  instead of DVE free-dim reductions over feature-major transposes.
- Edge We-matmuls for the first supers are emitted before the projections so
  the PE has work during the BN1 AllReduce window.
- k/v projections run first and kick the kv AllGather; q/skip projections and
  eaS prefetch overlap the collective.
- Attention loop: supers of TF=2 tiles; PSUM fully double-buffered
  (We 2x2 banks, q-broadcast 2 banks, agg 2 banks); ACT evacuates PSUM to
  bf16 SBUF so every bulk DVE op runs in 2x bf16 mode.
- MLP batched over 4 node blocks (N=512 matmuls).
"""

import os
import numpy as np
import ml_dtypes

import concourse.bass as bass
import concourse.bacc as bacc
import concourse.tile as tile
import concourse.mybir as mybir
from concourse.bass_utils import run_bass_kernel_spmd
from concourse.masks import make_identity

C = 8            # cores
N = 20000        # nodes
NL = N // C      # nodes per core
D = 384
H = 6
DHEAD = 64
DH = 2 * D       # mlp hidden
P = 128
NBLK = (NL + P - 1) // P      # 20 node blocks per core (last has 68)
TF = 2                        # tiles per super (PSUM double-buffer friendly)
NPRE = 8                      # supers whose We-matmuls are hoisted pre-proj
SCALE = 1.0 / np.sqrt(np.float32(DHEAD))
EPS = 1e-5

f32 = mybir.dt.float32
bf16 = mybir.dt.bfloat16
i32 = mybir.dt.int32
BF = ml_dtypes.bfloat16
AF = mybir.ActivationFunctionType
ALU = mybir.AluOpType


def _block_pb(b):
    return min(P, NL - b * P)


def _prep_host(x, edge_index, edge_attr, weights):
    """Shard + pad edges, build per-core input maps. Returns (in_maps, K, T)."""
    src = np.asarray(edge_index[0]).astype(np.int64)
    dst = np.asarray(edge_index[1]).astype(np.int64)
    x = np.asarray(x, dtype=np.float32)
    edge_attr = np.asarray(edge_attr, dtype=np.float32)

    cores = []
    cnt = np.zeros((C, NBLK), np.int64)
    for c in range(C):
        sel = (dst >= c * NL) & (dst < (c + 1) * NL)
        eids = np.nonzero(sel)[0]
        d_loc = (dst[eids] - c * NL).astype(np.int64)
        order = np.argsort(d_loc, kind="stable")
        eids = eids[order]
        d_loc = d_loc[order]
        s_glob = src[eids]
        blk = d_loc // P
        cnt[c] = np.bincount(blk, minlength=NBLK)
        cores.append((eids, d_loc, s_glob, blk))

    K = [max(1, int(-(-cnt[:, b].max() // P))) for b in range(NBLK)]
    T = sum(K)
    pad = (-T) % TF
    K[NBLK - 1] += pad
    T += pad
    tile_block = np.repeat(np.arange(NBLK), K)          # block id per tile
    blk_tile_start = np.concatenate([[0], np.cumsum(K)])[:NBLK]
    blk_edge_start = blk_tile_start * P

    # replicated weight tensors
    def chunks(w, nk):
        return np.stack([w[i * P:(i + 1) * P] for i in range(nk)]).astype(BF)

    (Wq, bq, Wk, bk, Wv, bv, We, Wskip, bskip,
     g1, b1, g2, b2, W1, bm1, W2, bm2) = weights

    def aug(w, b):
        a = np.zeros((4, P, w.shape[1]), np.float32)
        a[:3] = np.stack([w[i * P:(i + 1) * P] for i in range(3)])
        a[3, 0] = b
        return a.astype(BF)

    shared = {
        "Wq": aug(Wq, bq), "Wk": aug(Wk, bk), "Wv": aug(Wv, bv),
        "Wsk": aug(Wskip, bskip),
        "We": chunks(We, 3),
        "W1": aug(W1, bm1)[:3],                 # bias separately (bm1T)
        "bm1T": np.asarray(bm1, np.float32).reshape(H, P).T.copy(),
        "W2": chunks(W2, 6),
        "bm2": np.asarray(bm2, np.float32).reshape(1, D).astype(BF),
        "gb": np.stack([np.asarray(v, np.float32).reshape(3, P)
                        for v in (g1, b1, g2, b2)], axis=-1),  # [3, P, 4]
    }

    in_maps = []
    for c in range(C):
        eids, d_loc, s_glob, blk = cores[c]
        starts = np.searchsorted(blk, np.arange(NBLK))
        rank = np.arange(len(blk)) - starts[blk]
        pos = blk_edge_start[blk] + rank

        src_pad = np.zeros(T * P, np.int64)
        dst_pad = np.zeros(T * P, np.int64)
        valid = np.zeros(T * P, bool)
        src_pad[pos] = s_glob
        dst_pad[pos] = d_loc
        valid[pos] = True

        ea_pad = np.zeros((T * P, D), np.float32)
        ea_pad[pos] = edge_attr[eids]
        eaT = ea_pad.astype(BF).reshape(T, P, 3, P).transpose(0, 3, 2, 1)

        S = np.zeros((T * P, P), np.float32)
        tb = np.repeat(tile_block, P)
        S[np.nonzero(valid)[0], (dst_pad - tb * P)[valid]] = 1.0
        S = S.astype(BF).reshape(T, P, P)

        # combined [T, P, 5, P]: chunks 0-2 = eaT (partition=feature),
        # chunk 3 = S (partition=edge), chunk 4 = S^T (partition=node)
        ST = S.transpose(0, 2, 1)
        eaS = np.concatenate([eaT, S[:, :, None, :], ST[:, :, None, :]], axis=2)
        eaS = np.ascontiguousarray(eaS)

        kidx = np.where(valid, src_pad, 0)
        kidxT = np.ascontiguousarray(kidx.reshape(T, P).T).astype(np.int32)

        x_loc = np.ascontiguousarray(x[c * NL:(c + 1) * NL])
        xT_loc = np.ascontiguousarray(x_loc.T)

        m = {"x_loc": x_loc, "xT_loc": xT_loc, "eaS": eaS, "kidx": kidxT}
        m.update(shared)
        in_maps.append(m)
    return in_maps, K, T


def _build(K, T, zero_bias=False):
    NOCC = int(os.environ.get("KNOCC", "0"))
    LAG = 4                                    # agg matmul software pipeline
    LAGSTART = 10                              # first iter that emits aggs
    nc = bacc.Bacc("TRN2", target_bir_lowering=False, debug=False,
                   enable_asserts=False, num_devices=C)
    tile_block = np.repeat(np.arange(NBLK), K)
    blk_tile_start = np.concatenate([[0], np.cumsum(K)])[:NBLK]
    NSUP = T // TF

    # ------------- I/O -------------
    x_in = nc.dram_tensor("x_loc", [NL, D], f32, kind="ExternalInput")
    xT_in = nc.dram_tensor("xT_loc", [D, NL], f32, kind="ExternalInput")
    eaS_in = nc.dram_tensor("eaS", [T, P, 5, P], bf16, kind="ExternalInput")
    kidx_in = nc.dram_tensor("kidx", [P, T], i32, kind="ExternalInput")
    wq_in = nc.dram_tensor("Wq", [4, P, D], bf16, kind="ExternalInput")
    wk_in = nc.dram_tensor("Wk", [4, P, D], bf16, kind="ExternalInput")
    wv_in = nc.dram_tensor("Wv", [4, P, D], bf16, kind="ExternalInput")
    wsk_in = nc.dram_tensor("Wsk", [4, P, D], bf16, kind="ExternalInput")
    we_in = nc.dram_tensor("We", [3, P, D], bf16, kind="ExternalInput")
    w1_in = nc.dram_tensor("W1", [3, P, DH], bf16, kind="ExternalInput")
    bm1_in = nc.dram_tensor("bm1T", [P, H], f32, kind="ExternalInput")
    w2_in = nc.dram_tensor("W2", [6, P, D], bf16, kind="ExternalInput")
    bm2_in = nc.dram_tensor("bm2", [1, D], bf16, kind="ExternalInput")
    gb_in = nc.dram_tensor("gb", [3, P, 4], f32, kind="ExternalInput")
    out_dram = nc.dram_tensor("out", [NL, D], f32, kind="ExternalOutput")

    # ------------- internal DRAM -------------
    kv_part = nc.dram_tensor("kv_part", [NL, 2 * D], bf16)
    kv_full = nc.dram_tensor("kv_full", [C * NL, 2 * D], bf16,
                             addr_space="Shared")
    bn1_i = nc.dram_tensor("bn1_i", [P, 6], f32)
    bn1_o = nc.dram_tensor("bn1_o", [P, 6], f32, addr_space="Shared")
    bn2_i = nc.dram_tensor("bn2_i", [P, 6], f32)
    bn2_o = nc.dram_tensor("bn2_o", [P, 6], f32, addr_space="Shared")

    rg = [list(range(C))]

    with tile.TileContext(nc) as tc:
        with tc.tile_pool(name="const", bufs=1) as cp, \
             tc.tile_pool(name="persist", bufs=1) as pp, \
             tc.tile_pool(name="small", bufs=2) as sp:
            # ---- constants / weights ----
            idn_f = cp.tile([P, P], f32, tag="idn_f")
            make_identity(nc, idn_f[:])
            idn_b = cp.tile([P, P], bf16, tag="idn_b")
            make_identity(nc, idn_b[:])
            ones_b = cp.tile([1, NL], bf16, tag="ones_b")
            nc.vector.memset(ones_b[:], 1.0)
            ones_cf = cp.tile([P, 1], f32, tag="ones_cf")
            nc.vector.memset(ones_cf[:], 1.0)
            ones_cb = cp.tile([P, 1], bf16, tag="ones_cb")
            nc.vector.memset(ones_cb[:], 1.0)
            epst = cp.tile([P, 1], f32, tag="epst")
            nc.vector.memset(epst[:], EPS)
            wq_sb = cp.tile([P, 4, D], bf16, tag="wq")
            nc.sync.dma_start(wq_sb[:], wq_in.ap().rearrange("c p d -> p c d"))
            wk_sb = cp.tile([P, 4, D], bf16, tag="wk")
            nc.sync.dma_start(wk_sb[:], wk_in.ap().rearrange("c p d -> p c d"))
            wv_sb = cp.tile([P, 4, D], bf16, tag="wv")
            nc.sync.dma_start(wv_sb[:], wv_in.ap().rearrange("c p d -> p c d"))
            wsk_sb = cp.tile([P, 4, D], bf16, tag="wsk")
            nc.sync.dma_start(wsk_sb[:], wsk_in.ap().rearrange("c p d -> p c d"))
            we_sb = cp.tile([P, 3, D], bf16, tag="we")
            nc.sync.dma_start(we_sb[:], we_in.ap().rearrange("c p d -> p c d"))
            w1_sb = cp.tile([P, 3, DH], bf16, tag="w1")
            nc.sync.dma_start(w1_sb[:], w1_in.ap().rearrange("c p d -> p c d"))
            bm1_sb = cp.tile([P, H], f32, tag="bm1")
            nc.sync.dma_start(bm1_sb[:], bm1_in[:, :])
            w2_sb = cp.tile([P, 6, D], bf16, tag="w2")
            nc.sync.dma_start(w2_sb[:], w2_in.ap().rearrange("c p d -> p c d"))
            bm2_sb = cp.tile([1, D], bf16, tag="bm2")
            nc.sync.dma_start(bm2_sb[:], bm2_in[:, :])
            gb_sb = cp.tile([P, 3, 4], f32, tag="gb")
            nc.sync.dma_start(gb_sb[:], gb_in.ap().rearrange("c p j -> p c j"))
            kidx_sb = cp.tile([P, T], i32, tag="kidx")
            nc.sync.dma_start(kidx_sb[:], kidx_in[:, :])

            # ---- BN1 stats via PE column sums over node-major x blocks ----
            base = [pp.tile([P, D], bf16, tag=f"base{b}", name=f"base{b}")
                    for b in range(NBLK)]

            def pe_stats(src_tiles, stats_sb, sqtag):
                """Column sums + sums of squares of per-block node-major f32
                tiles -> stats_sb [P, 6] (chunk-major: sum,sumsq per chunk)."""
                with tc.tile_pool(name=sqtag, bufs=3) as qp_, \
                     tc.tile_pool(name=sqtag + "ps", bufs=1,
                                  space="PSUM") as qps:
                    s_ps = qps.tile([1, 512], f32, tag="s_ps", space="PSUM")
                    q_ps = qps.tile([1, 512], f32, tag="q_ps", space="PSUM")
                    for b in range(NBLK):
                        pb = _block_pb(b)
                        sq = qp_.tile([P, D], bf16, tag="sq")
                        nc.scalar.activation(sq[:pb, :], src_tiles[b][:pb, :],
                                             AF.Square)
                        nc.tensor.matmul(s_ps[0:1, 0:D], lhsT=ones_cb[:pb, :],
                                         rhs=src_tiles[b][:pb, :],
                                         start=(b == 0), stop=(b == NBLK - 1),
                                         skip_group_check=True)
                        nc.tensor.matmul(q_ps[0:1, 0:D], lhsT=ones_cb[:pb, :],
                                         rhs=sq[:pb, :],
                                         start=(b == 0), stop=(b == NBLK - 1),
                                         skip_group_check=True)
                    row = qp_.tile([1, 2, D], f32, tag="strow")
                    nc.scalar.activation(row[0:1, 0, :], s_ps[0:1, 0:D], AF.Copy)
                    nc.scalar.activation(row[0:1, 1, :], q_ps[0:1, 0:D], AF.Copy)
                    # transpose [1, 384]x2 -> [P, 6] (chunk c: sum, sumsq)
                    tp_ps = qps.tile([P, 6], f32, tag="tp_ps", space="PSUM")
                    for cch in range(3):
                        for s_ in range(2):
                            nc.tensor.transpose(
                                out=tp_ps[:, 2 * cch + s_:2 * cch + s_ + 1],
                                in_=row[0:1, s_, cch * P:(cch + 1) * P],
                                identity=idn_f[0:1, 0:1])
                    nc.vector.tensor_copy(stats_sb[:], tp_ps[:, :])

            bn1_sb = cp.tile([P, 6], f32, tag="bn1sb")
            for b in range(NBLK):
                pb = _block_pb(b)
                nc.gpsimd.dma_start(base[b][:pb, :],
                                    x_in[b * P:b * P + pb, :])
            pe_stats(base, bn1_sb, "bn1q")
            nc.sync.dma_start(bn1_i[:, :], bn1_sb[:])
            if NOCC:
                nc.sync.dma_start(bn1_o[:, :], bn1_i[:, :])
            else:
                nc.gpsimd.collective_compute(
                    "AllReduce", ALU.add, replica_groups=rg,
                    ins=[bn1_i.ap().opt()], outs=[bn1_o.ap().opt()])

            # ---- attention pools (created early: frontA prewarm uses them) ----
            att_ea_cm = tc.tile_pool(name="att_ea", bufs=14)
            eap = att_ea_cm.__enter__()
            att_ss_cm = tc.tile_pool(name="att_ss", bufs=12)
            ssp = att_ss_cm.__enter__()
            att_e_cm = tc.tile_pool(name="att_e", bufs=14)
            aep = att_e_cm.__enter__()
            att_qd_cm = tc.tile_pool(name="att_qd", bufs=12)
            qdp = att_qd_cm.__enter__()
            att_eps_cm = tc.tile_pool(name="att_eps", bufs=2, space="PSUM")
            pps = att_eps_cm.__enter__()

            ea_tiles = {}
            ss_tiles = {}
            e_tiles = {}

            def frontA(sg):
                """Edge-feature work with no BN/AllGather dependency:
                eaS DMA, We matmuls, ACT evac of e to bf16 SBUF."""
                t0 = TF * sg
                ea_t = eap.tile([P, TF, 3, P], bf16, tag="ea")
                nc.sync.dma_start(
                    ea_t[:], eaS_in.ap()[t0:t0 + TF, :, 0:3, :]
                    .rearrange("t p c x -> p t c x"))
                e_ps = pps.tile([P, TF, 512], f32, tag="e_ps", space="PSUM")
                for j in range(TF):
                    for kc in range(3):
                        nc.tensor.matmul(e_ps[:, j, 0:D],
                                         lhsT=ea_t[:, j, kc, :],
                                         rhs=we_sb[:, kc, :],
                                         start=(kc == 0), stop=(kc == 2),
                                         skip_group_check=True)
                e_sb = aep.tile([P, TF, D], bf16, tag="e_sb")
                nc.scalar.activation(e_sb[:], e_ps[:, :, 0:D], AF.Copy)
                ea_tiles[sg] = ea_t
                e_tiles[sg] = e_sb

            # pre-warm during the BN1 AllReduce window
            for sg in range(min(NPRE, NSUP)):
                frontA(sg)

            # ---- BN1 affine + hT ----
            st1 = cp.tile([P, 6], f32, tag="st1")
            nc.sync.dma_start(st1[:], bn1_o[:, :])

            def bn_affine(st, gcol, bcol, scn, bin_):
                sc_t = cp.tile([P, 3], f32, tag=scn)
                bi_t = cp.tile([P, 3], f32, tag=bin_)
                for c in range(3):
                    mean = sp.tile([P, 1], f32, tag="bn_mean")
                    nc.vector.tensor_scalar_mul(mean[:], st[:, 2 * c:2 * c + 1],
                                                1.0 / N)
                    var = sp.tile([P, 1], f32, tag="bn_var")
                    nc.vector.tensor_scalar_mul(var[:], st[:, 2 * c + 1:2 * c + 2],
                                                1.0 / N)
                    msq = sp.tile([P, 1], f32, tag="bn_msq")
                    nc.vector.tensor_tensor(out=msq[:], in0=mean[:], in1=mean[:],
                                            op=ALU.mult)
                    nc.vector.tensor_tensor(out=var[:], in0=var[:], in1=msq[:],
                                            op=ALU.subtract)
                    std = sp.tile([P, 1], f32, tag="bn_std")
                    nc.scalar.activation(std[:], var[:], AF.Sqrt, bias=epst[:, 0:1])
                    rstd = sp.tile([P, 1], f32, tag="bn_rstd")
                    nc.vector.reciprocal(rstd[:], std[:])
                    nc.vector.tensor_tensor(out=sc_t[:, c:c + 1], in0=rstd[:],
                                            in1=gb_sb[:, c, gcol:gcol + 1],
                                            op=ALU.mult)
                    ms = sp.tile([P, 1], f32, tag="bn_ms")
                    nc.vector.tensor_tensor(out=ms[:], in0=mean[:],
                                            in1=sc_t[:, c:c + 1], op=ALU.mult)
                    nc.vector.tensor_tensor(out=bi_t[:, c:c + 1],
                                            in0=gb_sb[:, c, bcol:bcol + 1],
                                            in1=ms[:], op=ALU.subtract)
                return sc_t, bi_t

            sc1, bi1 = bn_affine(st1, 0, 1, "sc1", "bi1")

            hp_cm = tc.tile_pool(name="bnh", bufs=1)
            hp = hp_cm.__enter__()
            hT = [hp.tile([P, NL], bf16, tag=f"hT{c}", name=f"hT{c}")
                  for c in range(3)]
            with tc.tile_pool(name="xTl", bufs=2) as xtp:
                for c in range(3):
                    xT_t = xtp.tile([P, NL], f32, tag="xT")
                    nc.sync.dma_start(xT_t[:], xT_in[c * P:(c + 1) * P, :])
                    nc.vector.tensor_scalar(out=hT[c][:], in0=xT_t[:],
                                            scalar1=sc1[:, c:c + 1],
                                            scalar2=bi1[:, c:c + 1],
                                            op0=ALU.mult, op1=ALU.add)

            # ---- k,v projections -> AllGather ----
            q_sb = [pp.tile([P, D], bf16, tag=f"q{b}", name=f"q{b}")
                    for b in range(NBLK)]
            with tc.tile_pool(name="proj", bufs=3) as jp, \
                 tc.tile_pool(name="projps", bufs=3, space="PSUM") as jpp:
                def proj_mm(ps, wsb, ns, pb):
                    for kc in range(3):
                        nc.tensor.matmul(ps[:pb, :], lhsT=hT[kc][:, ns],
                                         rhs=wsb[:, kc, :],
                                         start=(kc == 0),
                                         stop=(kc == 2 and zero_bias),
                                         skip_group_check=True)
                    if not zero_bias:
                        nc.tensor.matmul(ps[:pb, :], lhsT=ones_b[:, ns],
                                         rhs=wsb[0:1, 3, :],
                                         start=False, stop=True,
                                         skip_group_check=True)

                for b in range(NBLK):
                    pb = _block_pb(b)
                    ns = slice(b * P, b * P + pb)
                    kv_t = jp.tile([P, 2 * D], bf16, tag="kv_t")
                    for wsb, off in ((wk_sb, 0), (wv_sb, D)):
                        ps = jpp.tile([P, D], f32, tag="proj_ps", space="PSUM")
                        proj_mm(ps, wsb, ns, pb)
                        nc.vector.tensor_copy(kv_t[:pb, off:off + D],
                                              ps[:pb, :])
                    nc.sync.dma_start(kv_part[ns, :], kv_t[:pb, :])

                if NOCC:
                    for cc in range(C):
                        nc.sync.dma_start(
                            kv_full[cc * NL:(cc + 1) * NL, :], kv_part[:, :])
                else:
                    nc.gpsimd.collective_compute(
                        "AllGather", ALU.bypass, replica_groups=rg,
                        ins=[kv_part.ap().opt()], outs=[kv_full.ap().opt()])

                # ---- q, skip projections (overlap the AllGather) ----
                for b in range(NBLK):
                    pb = _block_pb(b)
                    ns = slice(b * P, b * P + pb)
                    for wsb, name in ((wq_sb, "q"), (wsk_sb, "s")):
                        ps = jpp.tile([P, D], f32, tag="proj_ps", space="PSUM")
                        proj_mm(ps, wsb, ns, pb)
                        if name == "q":
                            if pb < P:
                                nc.vector.memset(q_sb[b][:, :], 0.0)
                            nc.vector.tensor_copy(q_sb[b][:pb, :], ps[:pb, :])
                        else:
                            nc.vector.tensor_tensor(out=base[b][:pb, :],
                                                    in0=base[b][:pb, :],
                                                    in1=ps[:pb, :], op=ALU.add)
            hp_cm.__exit__(None, None, None)

            # ---- attention over edge supers (TF tiles, agg lagged by LAG) ----
            with tc.tile_pool(name="att_kv", bufs=3) as kvp, \
                 tc.tile_pool(name="att_sb", bufs=3) as asb, \
                 tc.tile_pool(name="att_rhs", bufs=LAG + 2) as rsp, \
                 tc.tile_pool(name="att_qps", bufs=2, space="PSUM") as qpp, \
                 tc.tile_pool(name="att_agg", bufs=2, space="PSUM") as agp, \
                 tc.tile_pool(name="fin", bufs=2) as fp:
                agg_ps = {}
                rhs_tiles = {}

                def frontB(sg):
                    t0 = TF * sg
                    e_sb = e_tiles[sg]
                    ss_t = ssp.tile([P, TF, 2, P], bf16, tag="ss")
                    nc.sync.dma_start(
                        ss_t[:], eaS_in.ap()[t0:t0 + TF, :, 3:5, :]
                        .rearrange("t p c x -> p t c x"))
                    ss_tiles[sg] = ss_t
                    kvsrc = kvp.tile([P, TF, 2 * D], bf16, tag="kvsrc")
                    for j in range(TF):
                        nc.gpsimd.indirect_dma_start(
                            out=kvsrc[:, j, :], out_offset=None,
                            in_=kv_full[:, :],
                            in_offset=bass.IndirectOffsetOnAxis(
                                ap=kidx_sb[:, t0 + j:t0 + j + 1], axis=0))

                    # q[dst] broadcast via S^T matmuls; ACT evacuates to bf16
                    qd_sb = qdp.tile([P, TF, D], bf16, tag="qd_sb")
                    for j in range(TF):
                        b = int(tile_block[t0 + j])
                        q_ps = qpp.tile([P, 512], f32, tag="q_ps", space="PSUM")
                        nc.tensor.matmul(q_ps[:, 0:D],
                                         lhsT=ss_t[:, j, 1, :],
                                         rhs=q_sb[b][:, :], start=True,
                                         stop=True, skip_group_check=True)
                        nc.scalar.activation(qd_sb[:, j, :], q_ps[:, 0:D],
                                             AF.Copy)

                    # logits = reduce_h(qd * (k[src] + e)) ; w = exp(scale*lg)
                    ke = asb.tile([P, TF, D], bf16, tag="ke")
                    nc.vector.tensor_tensor(out=ke[:], in0=kvsrc[:, :, 0:D],
                                            in1=e_sb[:], op=ALU.add)
                    prod = asb.tile([P, TF, D], bf16, tag="prod")
                    nc.vector.tensor_tensor(out=prod[:], in0=qd_sb[:],
                                            in1=ke[:], op=ALU.mult)
                    lg = asb.tile([P, TF, H], f32, tag="lg")
                    nc.vector.tensor_reduce(
                        out=lg[:],
                        in_=prod[:].rearrange("p t (h d) -> p t h d", h=H),
                        axis=mybir.AxisListType.X, op=ALU.add)
                    rhs = rsp.tile([P, TF, D + H], bf16, tag="rhs")
                    nc.scalar.activation(rhs[:, :, D:D + H], lg[:], AF.Exp,
                                         scale=float(SCALE))
                    # broadcast-expanded softmax weights via ACT (keeps the
                    # wve multiply in DVE 2x bf16 mode)
                    wexp = asb.tile([P, TF, D], bf16, tag="wexp")
                    nc.scalar.activation(
                        wexp[:].rearrange("p t (h d) -> p t h d", h=H),
                        lg[:, :, :, None].to_broadcast([P, TF, H, DHEAD]),
                        AF.Exp, scale=float(SCALE))
                    ve = asb.tile([P, TF, D], bf16, tag="ve")
                    nc.vector.tensor_tensor(out=ve[:], in0=kvsrc[:, :, D:2 * D],
                                            in1=e_sb[:], op=ALU.add)
                    nc.vector.tensor_tensor(out=rhs[:, :, 0:D], in0=ve[:],
                                            in1=wexp[:], op=ALU.mult)
                    rhs_tiles[sg] = rhs

                def emit_agg(sg):
                    t0 = TF * sg
                    ea_tiles.pop(sg)
                    e_tiles.pop(sg)
                    ss_t = ss_tiles.pop(sg)
                    rhs = rhs_tiles.pop(sg)
                    for j in range(TF):
                        t = t0 + j
                        b = int(tile_block[t])
                        first = (t == blk_tile_start[b])
                        last = (t == blk_tile_start[b] + K[b] - 1)
                        if first:
                            agg_ps[b] = agp.tile([P, 512], f32, tag="agg",
                                                 name=f"agg{b}", space="PSUM")
                        nc.tensor.matmul(agg_ps[b][:, 0:D + H],
                                         lhsT=ss_t[:, j, 0, :],
                                         rhs=rhs[:, j, :], start=first,
                                         stop=last, skip_group_check=True)
                        if last:
                            pb = _block_pb(b)
                            ag = agg_ps.pop(b)
                            ag_sb = fp.tile([P, D + H], f32, tag="ag_sb")
                            nc.scalar.activation(ag_sb[:pb, :],
                                                 ag[:pb, 0:D + H], AF.Copy)
                            dn = fp.tile([P, H], f32, tag="dn")
                            nc.vector.tensor_scalar_max(dn[:pb, :],
                                                        ag_sb[:pb, D:D + H],
                                                        1e-30)
                            rd = fp.tile([P, H], f32, tag="rd")
                            nc.vector.reciprocal(rd[:pb, :], dn[:pb, :])
                            at = fp.tile([P, D], bf16, tag="at")
                            nc.vector.tensor_tensor(
                                out=at[:pb, :].rearrange(
                                    "p (h d) -> p h d", h=H),
                                in0=ag_sb[:pb, 0:D].rearrange(
                                    "p (h d) -> p h d", h=H),
                                in1=rd[:pb, :, None].to_broadcast(
                                    [pb, H, DHEAD]),
                                op=ALU.mult)
                            nc.vector.tensor_tensor(out=base[b][:pb, :],
                                                    in0=base[b][:pb, :],
                                                    in1=at[:pb, :], op=ALU.add)

                pending = []
                for sg in range(NSUP):
                    if sg >= NPRE:
                        frontA(sg)
                    frontB(sg)
                    pending.append(sg)
                    if sg + 1 >= LAGSTART:
                        while pending and pending[0] <= sg - LAG:
                            emit_agg(pending.pop(0))
                while pending:
                    emit_agg(pending.pop(0))

            att_eps_cm.__exit__(None, None, None)
            att_qd_cm.__exit__(None, None, None)
            att_e_cm.__exit__(None, None, None)
            att_ss_cm.__exit__(None, None, None)
            att_ea_cm.__exit__(None, None, None)

            # ---- BN2 stats (PE, node-major) + AllReduce ----
            bn2_sb = cp.tile([P, 6], f32, tag="bn2sb")
            pe_stats(base, bn2_sb, "bn2q")
            nc.sync.dma_start(bn2_i[:, :], bn2_sb[:])
            if NOCC:
                nc.sync.dma_start(bn2_o[:, :], bn2_i[:, :])
            else:
                nc.gpsimd.collective_compute(
                    "AllReduce", ALU.add, replica_groups=rg,
                    ins=[bn2_i.ap().opt()], outs=[bn2_o.ap().opt()])

            # ---- x2 transposes (overlap the AllReduce) ----
            xp_cm = tc.tile_pool(name="x2tp", bufs=1)
            xp = xp_cm.__enter__()
            x2T = [xp.tile([P, NL], bf16, tag=f"x2T{c}", name=f"x2T{c}")
                   for c in range(3)]
            with tc.tile_pool(name="tp", bufs=3, space="PSUM") as tpp:
                for b in range(NBLK):
                    pb = _block_pb(b)
                    ns = slice(b * P, b * P + pb)
                    for dc in range(3):
                        tp_ps = tpp.tile([P, P], bf16, tag="tp_ps",
                                         space="PSUM")
                        nc.tensor.transpose(
                            out=tp_ps[:, :pb],
                            in_=base[b][:pb, dc * P:(dc + 1) * P],
                            identity=idn_b[:pb, :pb])
                        nc.scalar.activation(x2T[dc][:, ns], tp_ps[:, :pb],
                                             AF.Copy)

            st2 = cp.tile([P, 6], f32, tag="st2")
            nc.sync.dma_start(st2[:], bn2_o[:, :])
            sc2, bi2 = bn_affine(st2, 2, 3, "sc2", "bi2")

            h2T = [xp.tile([P, NL], bf16, tag=f"h2T{c}", name=f"h2T{c}")
                   for c in range(3)]
            for c in range(3):
                nc.vector.tensor_scalar(out=h2T[c][:], in0=x2T[c][:],
                                        scalar1=sc2[:, c:c + 1],
                                        scalar2=bi2[:, c:c + 1],
                                        op0=ALU.mult, op1=ALU.add)

            # ---- MLP batched over 4 node blocks (N=512) ----
            GB = 4
            with tc.tile_pool(name="mlp", bufs=2) as mp, \
                 tc.tile_pool(name="mlpout", bufs=2 * GB) as mop, \
                 tc.tile_pool(name="mlpps", bufs=3, space="PSUM") as mpp, \
                 tc.tile_pool(name="mlpps2", bufs=2, space="PSUM") as mpp2:
                for g in range(0, NBLK, GB):
                    out_tiles = {}
                    for b in range(g, min(g + GB, NBLK)):
                        out_tiles[b] = mop.tile([P, D], f32, tag="ot",
                                                name=f"ot{b}")
                    blks = list(range(g, min(g + GB, NBLK)))
                    n0 = g * P
                    ng = sum(_block_pb(b) for b in blks)
                    gs = slice(n0, n0 + ng)
                    gT = []
                    for oc in range(H):
                        m1 = mpp.tile([P, 512], f32, tag="m1", space="PSUM")
                        for kc in range(3):
                            nc.tensor.matmul(
                                m1[:, :ng],
                                lhsT=w1_sb[:, kc, oc * P:(oc + 1) * P],
                                rhs=h2T[kc][:, gs], start=(kc == 0),
                                stop=(kc == 2), skip_group_check=True)
                        g_t = mp.tile([P, 512], bf16, tag=f"gT{oc}")
                        nc.scalar.activation(g_t[:, :ng], m1[:, :ng], AF.Gelu,
                                             bias=bm1_sb[:, oc:oc + 1])
                        gT.append(g_t)
                    for dc in range(3):
                        m2 = mpp.tile([P, 512], f32, tag="m2", space="PSUM")
                        for oc in range(H):
                            nc.tensor.matmul(
                                m2[:, :ng],
                                lhsT=w2_sb[:, oc, dc * P:(dc + 1) * P],
                                rhs=gT[oc][:, :ng], start=(oc == 0),
                                stop=(oc == H - 1 and zero_bias),
                                skip_group_check=True)
                        if not zero_bias:
                            nc.tensor.matmul(
                                m2[:, :ng],
                                lhsT=bm2_sb[0:1, dc * P:(dc + 1) * P],
                                rhs=ones_b[:, gs], start=False,
                                stop=True, skip_group_check=True)
                        m2sb = mp.tile([P, 512], bf16, tag="m2sb")
                        nc.vector.tensor_copy(m2sb[:, :ng], m2[:, :ng])
                        for bi_, b in enumerate(blks):
                            pb = _block_pb(b)
                            m2tp = mpp2.tile([P, P], bf16, tag="m2tp",
                                             space="PSUM")
                            nc.tensor.transpose(
                                out=m2tp[:pb, :],
                                in_=m2sb[:, bi_ * P:bi_ * P + pb],
                                identity=idn_b[:])
                            ot = out_tiles[b]
                            nc.vector.tensor_tensor(
                                out=ot[:pb, dc * P:(dc + 1) * P],
                                in0=base[b][:pb, dc * P:(dc + 1) * P],
                                in1=m2tp[:pb, :], op=ALU.add)
                    for b in blks:
                        pb = _block_pb(b)
                        ns = slice(b * P, b * P + pb)
                        nc.sync.dma_start(out_dram[ns, :], out_tiles[b][:pb, :])
            xp_cm.__exit__(None, None, None)
    nc.compile()
    return nc


_CACHE = {}


def kernel(x, edge_index, edge_attr, g1, b1, Wq, bq, Wk, bk, Wv, bv, We,
           Wskip, bskip, g2, b2, W1, bm1, W2, bm2):
    weights = (np.asarray(Wq, np.float32), np.asarray(bq, np.float32),
               np.asarray(Wk, np.float32), np.asarray(bk, np.float32),
               np.asarray(Wv, np.float32), np.asarray(bv, np.float32),
               np.asarray(We, np.float32),
               np.asarray(Wskip, np.float32), np.asarray(bskip, np.float32),
               np.asarray(g1, np.float32), np.asarray(b1, np.float32),
               np.asarray(g2, np.float32), np.asarray(b2, np.float32),
               np.asarray(W1, np.float32), np.asarray(bm1, np.float32),
               np.asarray(W2, np.float32), np.asarray(bm2, np.float32))
    in_maps, K, T = _prep_host(x, edge_index, edge_attr, weights)
    zb = all(not np.any(np.asarray(v)) for v in (bq, bk, bv, bskip, bm2))
    key = (tuple(K), zb)
    if key not in _CACHE:
        _CACHE[key] = _build(K, T, zero_bias=zb)
    nc = _CACHE[key]
    res = run_bass_kernel_spmd(nc, in_maps, core_ids=list(range(C)))
    out = np.concatenate([res.results[c]["out"] for c in range(C)], axis=0)
    return out.astype(np.float32)


if __name__ == "__main__":
    import reference
    inputs = {k: np.asarray(v) for k, v in reference.setup_inputs().items()}
    got = kernel(**inputs)
    exp = np.asarray(reference.reference(**inputs))
    num = np.linalg.norm((got - exp).astype(np.float64))
    den = np.linalg.norm(exp.astype(np.float64))
    print("Relative error:", num / den)
